# revision 1
# baseline (speedup 1.0000x reference)
"""Multi-head self-attention (B=2, S=2048, D=1024, H=16, causal) on 8 trn2 cores.

Sharding: core c computes heads {2c, 2c+1} for both batches (column-parallel
QKV, row-parallel O). Each core returns a partial [4096, 1024] output
(attention output of its heads projected through its slice of o_proj);
the host sums the 8 partials.

Per-core kernel:
  - host supplies x pre-transposed (xT [1024, 4096]) and per-core weight
    slices pre-laid-out for SBUF.
  - projections (f32r matmuls, xT staged in token-halves): QT/KT
    [128, 2048] per batch stored bf16, V via VT + PE transpose stored bf16
    with a ones column so the AV matmul also produces the softmax
    denominator.
  - attention in transposed-score layout: scoresT[k, q] = K @ Q^T tiles
    (bf16, two heads packed on PE row groups), exp on ACT (scale 1/8
    fused) writing bf16, causal staircase skips invalid columns,
    triangular mask multiplies only diagonal blocks. Lag-1 software
    pipeline: AV for j-1 issues behind scores for j.
  - AV (bf16 in, fp32 accum): avT_aug[65, q] = V_aug^T @ expT; row 64 is
    the denominator. Raw results are copied to SBUF per qc so PSUM frees.
  - normalization (part1): r = exp(-ln(denom)); the denominator row is
    broadcast across 64 partitions with a f32r ones-outer-product matmul,
    ln/exp run on 64 lanes. Head 1 is shifted to partitions 64:128 with
    an SBUF->SBUF DMA so O contracts over all 128 dims in one chain.
  - O projection (part2) in f32r, K=128.
  - scheduling: part1/part2 are deferred and spread into later phases
    (batch 0's part1 into batch 1's projection phase where ACT is idle;
    O matmuls as PE fillers inside later qc j-loops) so the exp stream
    never stalls on the normalize chain.
"""

import os
import numpy as np
from contextlib import ExitStack

import concourse.bass as bass
import concourse.tile as tile
from concourse import bacc, mybir
from concourse.bass_utils import run_bass_kernel_spmd

F32R = mybir.dt.float32r
F32 = mybir.dt.float32
BF16 = mybir.dt.bfloat16
EXP = mybir.ActivationFunctionType.Exp
LN = mybir.ActivationFunctionType.Ln

B, S, D = 2, 2048, 1024
NT = B * S            # 4096 tokens total
NCORES = 8
SCALE = 0.125         # 1/sqrt(64)

_BUILT = None
LAST_RESULTS = None


def _build():
    nc = bacc.Bacc("TRN2", target_bir_lowering=False, debug=False,
                   num_devices=NCORES)
    xt_d = nc.dram_tensor("xt", [D, NT], F32R, kind="ExternalInput").ap()
    wq_d = nc.dram_tensor("wq", [128, D], F32R, kind="ExternalInput").ap()
    wk_d = nc.dram_tensor("wk", [128, D], F32R, kind="ExternalInput").ap()
    wv_d = nc.dram_tensor("wv", [128, D], F32R, kind="ExternalInput").ap()
    wo_d = nc.dram_tensor("wo", [128, 1024], F32R, kind="ExternalInput").ap()
    tri_d = nc.dram_tensor("tri", [128, 128], F32R, kind="ExternalInput").ap()
    id_d = nc.dram_tensor("ident", [128, 128], F32R, kind="ExternalInput").ap()
    ones_d = nc.dram_tensor("ones", [128, 64], F32, kind="ExternalInput").ap()
    out_d = nc.dram_tensor("out", [NT, D], BF16, kind="ExternalOutput").ap()

    with tile.TileContext(nc) as tc, ExitStack() as ctx:
        consts = ctx.enter_context(tc.tile_pool(name="consts", bufs=1))
        sb = ctx.enter_context(tc.tile_pool(name="sb", bufs=1))
        ps = ctx.enter_context(tc.tile_pool(name="ps", bufs=1, space="PSUM"))

        wq_t = consts.tile([128, D], F32R, tag="wq")
        nc.sync.dma_start(wq_t, wq_d)
        wk_t = consts.tile([128, D], F32R, tag="wk")
        nc.sync.dma_start(wk_t, wk_d)
        wv_t = consts.tile([128, D], F32R, tag="wv")
        nc.sync.dma_start(wv_t, wv_d)
        wo_t = consts.tile([128, 1024], F32R, tag="wo")
        nc.sync.dma_start(wo_t, wo_d)
        tri_t = consts.tile([128, 128], BF16, tag="tri")
        nc.gpsimd.dma_start(tri_t, tri_d)   # gpsimd DMA casts f32r->bf16
        id_t = consts.tile([128, 128], F32R, tag="ident")
        nc.sync.dma_start(id_t, id_d)
        # all-ones; row 64 is the lhsT of the f32r broadcast outer-product
        ones_t = consts.tile([65, 64], F32R, tag="ones")
        nc.gpsimd.dma_start(ones_t, ones_d[0:65, 0:64])

        # ---- deferred normalize (part1) and O projection (part2) ----
        def part1(b, qc, rawf):
            """r = exp(-ln(denominator)) broadcast over 64 partitions;
            avt_all[0:64] = h0 normalized, [64:128] = h1 (DMA-shifted)."""
            avt_all = sb.tile([128, 512], F32R, tag="avt", bufs=4,
                              name=f"avt{b}_{qc}")
            scl = sb.tile([128, 512], F32R, tag="scl", bufs=2,
                          name=f"scl{b}_{qc}")
            scl2 = sb.tile([64, 512], F32R, tag="scl2", bufs=2,
                           name=f"scl2_{b}_{qc}")
            lnr = sb.tile([64, 512], F32, tag="lnr", bufs=4,
                          name=f"lnr{b}_{qc}")
            lnr2 = sb.tile([64, 512], F32, tag="lnr", bufs=4,
                           name=f"lnr2{b}_{qc}")
            rawsh = sb.tile([128, 512], F32R, tag="rawsh", bufs=2,
                            name=f"rawsh{b}_{qc}")
            cs = slice(512 * qc, 512 * (qc + 1))
            nc.sync.dma_start(rawsh[64:128, :], rawf[1][0:64, cs])

            bc0 = ps.tile([64, 512], F32, tag="mm", bufs=5,
                          name=f"bc0_{b}_{qc}")
            nc.tensor.matmul(bc0, lhsT=ones_t[64:65, :],
                             rhs=rawf[0][64:65, cs], start=True, stop=True)
            bc1 = ps.tile([64, 512], F32, tag="mm", bufs=5,
                          name=f"bc1_{b}_{qc}")
            nc.tensor.matmul(bc1, lhsT=ones_t[64:65, :],
                             rhs=rawf[1][64:65, cs], start=True, stop=True)
            nc.scalar.activation(lnr[0:64, :], bc0, LN)
            nc.scalar.activation(lnr2[0:64, :], bc1, LN)
            nc.scalar.activation(scl[0:64, :], lnr[0:64, :], EXP, scale=-1.0)
            nc.scalar.activation(scl2, lnr2[0:64, :], EXP, scale=-1.0)
            nc.sync.dma_start(scl[64:128, :], scl2)
            nc.vector.tensor_mul(avt_all[0:64, :], rawf[0][0:64, cs],
                                 scl[0:64, :])
            nc.vector.tensor_mul(avt_all[64:128, :], rawsh[64:128, :],
                                 scl[64:128, :])
            return avt_all

        def part2_unit(b, qc, avt_all, tt):
            """One token-tile of the O projection: 2 matmuls + copy + DMA."""
            ost = sb.tile([128, 1024], BF16, tag="ost", bufs=2,
                          name=f"ost{b}_{qc}_{tt}")
            for chv in range(2):
                op = ps.tile([128, 512], F32, tag="mm", bufs=5,
                             name=f"op{b}_{qc}_{tt}_{chv}")
                nc.tensor.matmul(
                    op,
                    lhsT=avt_all[:, 128 * tt:128 * (tt + 1)],
                    rhs=wo_t[:, 512 * chv:512 * (chv + 1)],
                    start=True, stop=True)
                nc.vector.tensor_copy(ost[:, 512 * chv:512 * (chv + 1)], op)
            row0 = S * b + 512 * qc + 128 * tt
            nc.sync.dma_start(out_d[row0:row0 + 128, :], ost)

        # schedule state
        avt_ready = {}            # (b, qc) -> avt_all tile
        rawf_of = {}              # b -> rawf pair

        def attention(b, qt, kt, vg, rawf, part1_at_j1, fillers):
            """fillers: per-qc list of (b, qc, tt) O-units to spread."""
            for qc in range(4):
                njt = 4 * qc + 4
                avps = [ps.tile([128, 512], F32, tag="av", bufs=2,
                                name=f"avps{b}_{qc}_{h}")
                        for h in range(2)]
                fl = fillers[qc]
                nfl = len(fl)
                p1 = part1_at_j1[qc]
                pend = []

                def do_av(j, ets):
                    vs = max(0, 128 * (j - 4 * qc))
                    for h in range(2):
                        nc.tensor.matmul(
                            avps[h][0:65, vs:512],
                            lhsT=vg[h][:, j, 0:65],
                            rhs=ets[h][:, vs:512],
                            start=(j == 0), stop=(j == njt - 1),
                            skip_group_check=True)

                for j in range(njt):
                    vs = max(0, 128 * (j - 4 * qc))
                    ets = []
                    for h in range(2):
                        sc = ps.tile([128, 512], F32, tag="mm", bufs=5)
                        nc.tensor.matmul(
                            sc[:, vs:512],
                            lhsT=kt[64 * h:64 * (h + 1), 128 * j:128 * (j + 1)],
                            rhs=qt[64 * h:64 * (h + 1), 512 * qc + vs:512 * (qc + 1)],
                            start=True, stop=True)
                        et = sb.tile([128, 512], BF16, tag=f"et{h}", bufs=4)
                        nc.scalar.activation(et[:, vs:512], sc[:, vs:512],
                                             EXP, scale=SCALE)
                        if j >= 4 * qc:
                            nc.vector.tensor_mul(et[:, vs:vs + 128],
                                                 et[:, vs:vs + 128], tri_t)
                        ets.append(et)
                    pend.append((j, ets))
                    if len(pend) > 2:   # lag-2: AV issues two iterations behind
                        do_av(*pend.pop(0))
                    if j == 1 and p1 is not None:
                        avt_ready[p1] = part1(p1[0], p1[1], rawf_of[p1[0]])
                    # spread O-unit fillers across the loop
                    k0 = nfl * j // njt
                    k1 = nfl * (j + 1) // njt
                    for k in range(k0, k1):
                        fb, fqc, ftt = fl[k]
                        part2_unit(fb, fqc, avt_ready[(fb, fqc)], ftt)
                for args in pend:
                    do_av(*args)
                for h in range(2):
                    nc.vector.tensor_copy(rawf[h][:, 512 * qc:512 * (qc + 1)],
                                          avps[h][0:65, :])

        for b in range(B):
            # ---------- projections (token-halves to limit xt residency) --
            xth = []
            for half in range(2):
                row = []
                for k in range(8):
                    xk = sb.tile([128, S // 2], F32R, tag="xt", bufs=9,
                                 name=f"xt{b}_{half}_{k}")
                    nc.sync.dma_start(
                        xk, xt_d[128 * k:128 * (k + 1),
                                 S * b + 1024 * half:S * b + 1024 * (half + 1)])
                    row.append(xk)
                xth.append(row)

            def project(w_t, tag, dt, vbufs=2):
                dst = sb.tile([128, S], dt, tag=tag, bufs=vbufs)
                for chk in range(4):
                    half, sub = chk // 2, chk % 2
                    pp = ps.tile([128, 512], F32, tag="mm", bufs=5)
                    for k in range(8):
                        nc.tensor.matmul(
                            pp, lhsT=w_t[:, 128 * k:128 * (k + 1)],
                            rhs=xth[half][k][:, 512 * sub:512 * (sub + 1)],
                            start=(k == 0), stop=(k == 7))
                    nc.vector.tensor_copy(dst[:, 512 * chk:512 * (chk + 1)], pp)
                return dst

            qt = project(wq_t, "qt", BF16)
            kt = project(wk_t, "kt", BF16)
            vt = project(wv_t, "vt", F32R, vbufs=1)

            # V in token-partition layout, + ones column for the denominator
            vg = []
            for h in range(2):
                vgh = sb.tile([128, 16, 66], BF16, tag=f"vg{h}", bufs=2)
                nc.gpsimd.dma_start(vgh[:, :, 64:65], ones_d[:, 0:16])
                vg.append(vgh)
            for j in range(16):
                tp = ps.tile([128, 128], F32R, tag="mm", bufs=5)
                nc.tensor.transpose(tp, vt[:, 128 * j:128 * (j + 1)], id_t)
                nc.vector.tensor_copy(vg[0][:, j, 0:64], tp[:, 0:64])
                nc.vector.tensor_copy(vg[1][:, j, 0:64], tp[:, 64:128])

            rawf = [sb.tile([65, S], F32R, tag=f"rawfull{h}", bufs=2,
                            name=f"rawf{b}_{h}")
                    for h in range(2)]
            rawf_of[b] = rawf

            if b == 0:
                attention(b, qt, kt, vg, rawf,
                          part1_at_j1=[None] * 4, fillers=[[], [], [], []])
            else:
                # batch 0's normalize + O projection land here: the
                # projection phase has ACT/PE/DVE slack for all of it
                for qc in range(4):
                    avt_ready[(0, qc)] = part1(0, qc, rawf_of[0])
                # b1 attention: all deferred O-units spread evenly (~0.7/j)
                units = ([(0, q, t) for q in range(4) for t in range(4)] +
                         [(1, 0, t) for t in range(4)] +
                         [(1, 1, t) for t in range(4)] +
                         [(1, 2, t) for t in range(4)])
                attention(b, qt, kt, vg, rawf,
                          part1_at_j1=[None, (1, 0), (1, 1), (1, 2)],
                          fillers=[units[0:2], units[2:8], units[8:17],
                                   units[17:28]])
        # tail: the last pieces that cannot hide anywhere
        avt_ready[(1, 3)] = part1(1, 3, rawf_of[1])
        for tt in range(4):
            part2_unit(1, 3, avt_ready[(1, 3)], tt)
    nc.compile()
    return nc


def _get_built():
    global _BUILT
    if _BUILT is None:
        _BUILT = _build()
    return _BUILT


def _host_inputs(x, q_proj, k_proj, v_proj, o_proj):
    xth = np.ascontiguousarray(x.reshape(NT, D).T)
    tri = np.triu(np.ones((128, 128), dtype=np.float32))
    ident = np.eye(128, dtype=np.float32)

    def wslice(w, c):
        # [p, 8k x 128m]: w_sb[p, 128k+m] = w[128c+m, 128k+p]
        a = w[128 * c:128 * (c + 1)].reshape(128, 8, 128)
        return np.ascontiguousarray(a.transpose(2, 1, 0).reshape(128, D))

    in_maps = []
    for c in range(NCORES):
        wo = np.ascontiguousarray(o_proj[:, 128 * c:128 * (c + 1)].T)
        in_maps.append(dict(
            xt=xth, wq=wslice(q_proj, c), wk=wslice(k_proj, c),
            wv=wslice(v_proj, c), wo=wo, tri=tri, ident=ident,
            ones=np.ones((128, 64), dtype=np.float32)))
    return in_maps


def kernel(**inputs):
    x = np.asarray(inputs["x"], dtype=np.float32)
    q_proj = np.asarray(inputs["q_proj"], dtype=np.float32)
    k_proj = np.asarray(inputs["k_proj"], dtype=np.float32)
    v_proj = np.asarray(inputs["v_proj"], dtype=np.float32)
    o_proj = np.asarray(inputs["o_proj"], dtype=np.float32)

    in_maps = _host_inputs(x, q_proj, k_proj, v_proj, o_proj)
    nc = _get_built()
    global LAST_RESULTS
    LAST_RESULTS = run_bass_kernel_spmd(
        nc, in_maps, core_ids=list(range(NCORES)),
        trace=bool(os.environ.get("KERNEL_TRACE")))
    acc = np.asarray(LAST_RESULTS.results[0]["out"]).astype(np.float32)
    for c in range(1, NCORES):
        acc += np.asarray(LAST_RESULTS.results[c]["out"]).astype(np.float32)
    return acc.reshape(B, S, D)



# revision 4
# speedup vs baseline: 1.2980x; 1.2980x over previous
"""Multi-head self-attention (B=2, S=2048, D=1024, H=16, causal) on 8 trn2 cores.

Sharding: core c = (batch b=c//4, head-group g=c%4 of 4 heads = dims
256g:256g+256). Column-parallel QKV, row-parallel O; each core returns a
partial [2048, 1024] output for its batch; host sums 4 partials per batch.

Per-core kernel (all matmul inputs bf16):
  - projections from xt [1024, 2048] bf16: QT/KT [128, 2, 2048] bf16
    (2 dim-groups x 2048 tokens); V computed directly in token-major
    layout vg[128 tok, 16 tile, 4 head, 65] with a ones column per head
    so the AV matmul also produces the softmax denominator.
  - attention in transposed-score layout: scoresT[k, q] = K @ Q^T tiles,
    exp on ACT (scale 1/8 fused) writing bf16, causal staircase skips
    invalid columns, triangular mask multiplies only diagonal blocks.
    Lag-1 software pipeline: AV for j-1 issues behind scores for j.
  - AV (bf16, fp32 accum): av[h][65, q]; row 64 is the denominator.
  - normalize: raw copy to SBUF (frees PSUM), denominator broadcast via
    f32r ones outer-product matmul, reciprocal_approx_fast on DVE (no Ln
    -> no ACT table switches), multiply to bf16 pairs; odd heads shifted
    to partitions 64:128 by SBUF-SBUF DMA so O contracts 128 dims/pair.
  - O projection: per 128-token tile, 2 chained matmuls (pairA, pairB)
    per 512-col chunk; drains + output DMA per token tile.
  - scheduling: normalize + O matmuls spread as PE fillers inside later
    qc j-loops (the exp stream on ACT is the attention-phase pacer).
"""

import os
import numpy as np
from contextlib import ExitStack

import ml_dtypes

import concourse.bass as bass
import concourse.tile as tile
from concourse import bacc, mybir
from concourse.bass_utils import run_bass_kernel_spmd

F32R = mybir.dt.float32r
F32 = mybir.dt.float32
BF16 = mybir.dt.bfloat16
EXP = mybir.ActivationFunctionType.Exp

B, S, D = 2, 2048, 1024
NCORES = 8
SCALE = 0.125         # 1/sqrt(64)
NH = 4                # heads per core

_BUILT = None
LAST_RESULTS = None


def _build():
    nc = bacc.Bacc("TRN2", target_bir_lowering=False, debug=False,
                   num_devices=NCORES)
    xt_d = nc.dram_tensor("xt", [D, S], BF16, kind="ExternalInput").ap()
    wq_d = nc.dram_tensor("wq", [128, 2048], BF16, kind="ExternalInput").ap()
    wk_d = nc.dram_tensor("wk", [128, 2048], BF16, kind="ExternalInput").ap()
    wvt_d = nc.dram_tensor("wvt", [128, 2048], BF16, kind="ExternalInput").ap()
    wo_d = nc.dram_tensor("wo", [128, 2048], BF16, kind="ExternalInput").ap()
    tri_d = nc.dram_tensor("tri", [128, 128], BF16, kind="ExternalInput").ap()
    onesb_d = nc.dram_tensor("onesb", [128, 64], BF16,
                             kind="ExternalInput").ap()
    onesr_d = nc.dram_tensor("onesr", [1, 64], F32R, kind="ExternalInput").ap()
    out_d = nc.dram_tensor("out", [S, D], BF16, kind="ExternalOutput").ap()

    with tile.TileContext(nc) as tc, ExitStack() as ctx:
        consts = ctx.enter_context(tc.tile_pool(name="consts", bufs=1))
        sb = ctx.enter_context(tc.tile_pool(name="sb", bufs=1))
        ps = ctx.enter_context(tc.tile_pool(name="ps", bufs=1, space="PSUM"))

        wq_t = consts.tile([128, 2048], BF16, tag="wq")
        nc.sync.dma_start(wq_t, wq_d)
        # xt quarters: xtq[k][tc] = xt[128k:128k+128, 512tc:512tc+512]
        xtq = []
        for k in range(8):
            row = []
            for tc2 in range(2):
                xk = sb.tile([128, 512], BF16, tag="xtq", bufs=32,
                             name=f"xtq{k}_{tc2}")
                nc.sync.dma_start(
                    xk, xt_d[128 * k:128 * (k + 1), 512 * tc2:512 * (tc2 + 1)])
                row.append(xk)
            xtq.append(row)
        wk_t = consts.tile([128, 2048], BF16, tag="wk")
        nc.sync.dma_start(wk_t, wk_d)
        for k in range(8):
            for tc2 in range(2, 4):
                xk = sb.tile([128, 512], BF16, tag="xtq", bufs=32,
                             name=f"xtq{k}_{tc2}")
                nc.sync.dma_start(
                    xk, xt_d[128 * k:128 * (k + 1), 512 * tc2:512 * (tc2 + 1)])
                xtq[k].append(xk)
        wvt_t = consts.tile([128, 2048], BF16, tag="wvt")
        nc.sync.dma_start(wvt_t, wvt_d)
        wo_t = consts.tile([128, 2048], BF16, tag="wo")
        nc.sync.dma_start(wo_t, wo_d)
        tri_t = consts.tile([128, 128], BF16, tag="tri")
        nc.gpsimd.dma_start(tri_t, tri_d)
        onesb_t = consts.tile([128, 64], BF16, tag="onesb")
        nc.gpsimd.dma_start(onesb_t, onesb_d)
        # ones row placed at partition 64 so the denominator-broadcast
        # matmul's lhsT shares the den row's base partition
        onesr_t = consts.tile([65, 64], F32R, tag="onesr")
        nc.gpsimd.dma_start(onesr_t[64:65, :], onesr_d)

        # ---------------- projections ----------------
        qt = sb.tile([128, 2, 2048], BF16, tag="qt")
        kt = sb.tile([128, 2, 2048], BF16, tag="kt")
        vg = sb.tile([128, 16, NH, 65], BF16, tag="vg")
        # ones columns for the denominator rows (one strided DMA)
        nc.gpsimd.dma_start(vg[:, :, :, 64:65], onesb_d[:, 0:64])

        def project_qk(w_t, dst):
            for gg in range(2):
                for tc2 in range(4):
                    pp = ps.tile([128, 512], F32, tag="mm", bufs=4)
                    for k in range(8):
                        nc.tensor.matmul(
                            pp,
                            lhsT=w_t[:, 1024 * gg + 128 * k:
                                     1024 * gg + 128 * (k + 1)],
                            rhs=xtq[k][tc2],
                            start=(k == 0), stop=(k == 7))
                    nc.scalar.activation(
                        dst[:, gg, 512 * tc2:512 * (tc2 + 1)], pp,
                        mybir.ActivationFunctionType.Copy)

        project_qk(wq_t, qt)
        project_qk(wk_t, kt)
        for j in range(16):
            vp = ps.tile([128, 256], F32, tag="mm", bufs=4)
            for k in range(8):
                nc.tensor.matmul(
                    vp,
                    lhsT=xtq[k][j // 4][:, 128 * (j % 4):128 * (j % 4 + 1)],
                    rhs=wvt_t[:, 256 * k:256 * (k + 1)],
                    start=(k == 0), stop=(k == 7))
            nc.vector.tensor_copy(vg[:, j, :, 0:64], vp)

        # ---------------- deferred normalize (part1) / O proj (part2) ----
        avps = {}             # h -> current psum accumulator
        raw_of = {}           # qc -> [raw tiles per h]
        pair_of = {}          # qc -> (pairA, pairB)

        def part1_raw(qc):
            """Drain AV psum to SBUF right at the qc boundary (frees banks)."""
            raws = []
            for h in range(NH):
                raw = sb.tile([65, 512], F32R, tag="raw", bufs=8,
                              name=f"raw{qc}_{h}")
                nc.vector.tensor_copy(raw, avps[h][0:65, :])
                raws.append(raw)
            raw_of[qc] = raws

        def part1_norm(qc):
            """reciprocal of denominators, broadcast, multiply to bf16."""
            raws = raw_of[qc]
            pairs = []
            for p in range(2):
                pair = sb.tile([128, 512], BF16, tag="pair", bufs=4,
                               name=f"pair{qc}_{p}")
                pairs.append(pair)
            for h in range(NH):
                bc = ps.tile([64, 512], F32, tag="mm", bufs=4,
                             name=f"bc{qc}_{h}")
                nc.tensor.matmul(bc, lhsT=onesr_t[64:65, 0:64],
                                 rhs=raws[h][64:65, :],
                                 start=True, stop=True,
                                 skip_group_check=True)
                rcp = sb.tile([64, 512], F32, tag="rcp", bufs=4,
                              name=f"rcp{qc}_{h}")
                nc.vector.reciprocal_approx_fast(rcp, bc)
                if h % 2 == 0:
                    nc.vector.tensor_mul(pairs[h // 2][0:64, :],
                                         raws[h][0:64, :], rcp)
                else:
                    tmp = sb.tile([64, 512], BF16, tag="tmp", bufs=2,
                                  name=f"tmp{qc}_{h}")
                    nc.vector.tensor_mul(tmp, raws[h][0:64, :], rcp)
                    nc.sync.dma_start(pairs[h // 2][64:128, :], tmp)
            pair_of[qc] = pairs

        def part2_unit(qc, tt):
            """One 128-token tile of the O projection."""
            pairs = pair_of[qc]
            ost = sb.tile([128, 1024], BF16, tag="ost", bufs=2,
                          name=f"ost{qc}_{tt}")
            for chv in range(2):
                op = ps.tile([128, 512], F32, tag="mm", bufs=4,
                             name=f"op{qc}_{tt}_{chv}")
                nc.tensor.matmul(
                    op, lhsT=pairs[0][:, 128 * tt:128 * (tt + 1)],
                    rhs=wo_t[:, 512 * chv:512 * (chv + 1)],
                    start=True, stop=False, skip_group_check=True)
                nc.tensor.matmul(
                    op, lhsT=pairs[1][:, 128 * tt:128 * (tt + 1)],
                    rhs=wo_t[:, 1024 + 512 * chv:1024 + 512 * (chv + 1)],
                    start=False, stop=True, skip_group_check=True)
                nc.vector.tensor_copy(ost[:, 512 * chv:512 * (chv + 1)], op)
            row0 = 512 * qc + 128 * tt
            nc.sync.dma_start(out_d[row0:row0 + 128, :], ost)

        # ---------------- attention ----------------
        def attention(qc, fillers):
            """fillers: list of callables spread across the j-loop."""
            njt = 4 * qc + 4
            for h in range(NH):
                avps[h] = ps.tile([128, 512], F32, tag=f"av{h}", bufs=1,
                                  name=f"avps{qc}_{h}")
            nfl = len(fillers)
            pend = []

            def do_av(j, ets):
                vs = max(0, 128 * (j - 4 * qc))
                for h in range(NH):
                    nc.tensor.matmul(
                        avps[h][0:65, vs:512],
                        lhsT=vg[:, j, h, 0:65],
                        rhs=ets[h][:, vs:512],
                        start=(j == 0), stop=(j == njt - 1),
                        skip_group_check=True)

            for j in range(njt):
                vs = max(0, 128 * (j - 4 * qc))
                ets = []
                for h in range(NH):
                    hp = 64 * (h % 2)
                    gg = h // 2
                    sc = ps.tile([128, 512], F32, tag="mm", bufs=4)
                    nc.tensor.matmul(
                        sc[:, vs:512],
                        lhsT=kt[hp:hp + 64, gg, 128 * j:128 * (j + 1)],
                        rhs=qt[hp:hp + 64, gg,
                               512 * qc + vs:512 * (qc + 1)],
                        start=True, stop=True)
                    et = sb.tile([128, 512], BF16, tag=f"et{h}", bufs=3)
                    nc.scalar.activation(et[:, vs:512], sc[:, vs:512],
                                         EXP, scale=SCALE)
                    if j >= 4 * qc:
                        nc.vector.tensor_mul(et[:, vs:vs + 128],
                                             et[:, vs:vs + 128], tri_t)
                    ets.append(et)
                pend.append((j, ets))
                if len(pend) > 1:   # lag-1 AV pipeline
                    do_av(*pend.pop(0))
                k0 = nfl * j // njt
                k1 = nfl * (j + 1) // njt
                for k in range(k0, k1):
                    fillers[k]()
            for args in pend:
                do_av(*args)
            part1_raw(qc)

        attention(0, [])
        attention(1, [lambda: part1_norm(0),
                      lambda: part2_unit(0, 0),
                      lambda: part2_unit(0, 1)])
        attention(2, [lambda: part2_unit(0, 2),
                      lambda: part2_unit(0, 3),
                      lambda: part1_norm(1),
                      lambda: part2_unit(1, 0),
                      lambda: part2_unit(1, 1),
                      lambda: part2_unit(1, 2)])
        attention(3, [lambda: part2_unit(1, 3),
                      lambda: part1_norm(2),
                      lambda: part2_unit(2, 0),
                      lambda: part2_unit(2, 1),
                      lambda: part2_unit(2, 2),
                      lambda: part2_unit(2, 3)])
        # tail
        part1_norm(3)
        for tt in range(4):
            part2_unit(3, tt)
    nc.compile()
    return nc


def _get_built():
    global _BUILT
    if _BUILT is None:
        _BUILT = _build()
    return _BUILT


def _host_inputs(x, q_proj, k_proj, v_proj, o_proj):
    bf = ml_dtypes.bfloat16
    xt = [np.ascontiguousarray(x[b].T.astype(bf)) for b in range(B)]
    tri = np.triu(np.ones((128, 128), dtype=np.float32)).astype(bf)
    onesb = np.ones((128, 64), dtype=np.float32).astype(bf)
    onesr = np.ones((1, 64), dtype=np.float32)

    def wqk(w, g):
        # [fp, 1024*gg + 128*k + m] = w[256g+128gg+m, 128k+fp]
        a = w[256 * g:256 * (g + 1)].reshape(2, 128, 8, 128)
        return np.ascontiguousarray(
            a.transpose(3, 0, 2, 1).reshape(128, 2048).astype(bf))

    def wvt(w, g):
        # [fp, 256*k + vd] = w[256g+vd, 128k+fp]
        a = w[256 * g:256 * (g + 1)].reshape(256, 8, 128)
        return np.ascontiguousarray(
            a.transpose(2, 1, 0).reshape(128, 2048).astype(bf))

    def wo_s(w, g):
        # [dp, 1024*dd + o] = o_proj[o, 256g+128dd+dp]
        a = w[:, 256 * g:256 * (g + 1)].reshape(1024, 2, 128)
        return np.ascontiguousarray(
            a.transpose(2, 1, 0).reshape(128, 2048).astype(bf))

    in_maps = []
    for c in range(NCORES):
        b, g = c // 4, c % 4
        in_maps.append(dict(
            xt=xt[b], wq=wqk(q_proj, g), wk=wqk(k_proj, g),
            wvt=wvt(v_proj, g), wo=wo_s(o_proj, g), tri=tri,
            onesb=onesb, onesr=onesr))
    return in_maps


def kernel(**inputs):
    x = np.asarray(inputs["x"], dtype=np.float32)
    q_proj = np.asarray(inputs["q_proj"], dtype=np.float32)
    k_proj = np.asarray(inputs["k_proj"], dtype=np.float32)
    v_proj = np.asarray(inputs["v_proj"], dtype=np.float32)
    o_proj = np.asarray(inputs["o_proj"], dtype=np.float32)

    in_maps = _host_inputs(x, q_proj, k_proj, v_proj, o_proj)
    nc = _get_built()
    global LAST_RESULTS
    LAST_RESULTS = run_bass_kernel_spmd(
        nc, in_maps, core_ids=list(range(NCORES)),
        trace=bool(os.environ.get("KERNEL_TRACE")))
    out = np.zeros((B, S, D), dtype=np.float32)
    for c in range(NCORES):
        out[c // 4] += np.asarray(
            LAST_RESULTS.results[c]["out"]).astype(np.float32)
    return out


# revision 5
# speedup vs baseline: 1.4413x; 1.1105x over previous
"""Multi-head self-attention (B=2, S=2048, D=1024, H=16, causal) on 8 trn2 cores.

Sharding: core c = (batch b=c//4, head-group g=c%4 of 4 heads = dims
256g:256g+256). Column-parallel QKV, row-parallel O; each core returns a
partial [2048, 1024] output for its batch; host sums 4 partials per batch.

Per-core kernel (all matmul inputs bf16):
  - projections from xt [1024, 2048] bf16: QT/KT [128, 2, 2048] bf16;
    V computed directly in token-major layout vg[128, 16, 4, 65] with a
    ones column per head so AV also produces the softmax denominator.
  - attention in transposed-score layout: scoresT[k, q] = K @ Q^T tiles,
    exp on ACT (scale 1/8 fused) to bf16, causal staircase skips invalid
    columns, triangular mask on diagonal blocks only. Lag-1 AV pipeline.
  - software pipeline: only the first token-chunk of Q/K and V tiles 0-3
    are projected up front; the remaining projection chains run as PE
    fillers inside the attention j-loops (which are paced by the exp
    stream on ACT), as do the deferred normalize + O-projection units.
  - normalize: raw AV copy to SBUF (frees PSUM), denominator broadcast
    via f32r ones outer-product matmul, reciprocal_approx_fast on DVE
    (no Ln -> no ACT table switches), multiply to bf16 pairs; odd heads
    DMA-shifted to partitions 64:128 so O contracts 128 dims per pair.
  - O projection: 2 chained matmuls per 512-col chunk; per-chunk drain
    and output DMA.
"""

import os
import numpy as np
from contextlib import ExitStack

import ml_dtypes

import concourse.bass as bass
import concourse.tile as tile
from concourse import bacc, mybir
from concourse.bass_utils import run_bass_kernel_spmd

F32R = mybir.dt.float32r
F32 = mybir.dt.float32
BF16 = mybir.dt.bfloat16
EXP = mybir.ActivationFunctionType.Exp
COPY = mybir.ActivationFunctionType.Copy

B, S, D = 2, 2048, 1024
NCORES = 8
SCALE = 0.125         # 1/sqrt(64)
NH = 4                # heads per core

_BUILT = None
LAST_RESULTS = None


def _build():
    nc = bacc.Bacc("TRN2", target_bir_lowering=False, debug=False,
                   num_devices=NCORES)
    xt_d = nc.dram_tensor("xt", [D, S], BF16, kind="ExternalInput").ap()
    wq_d = nc.dram_tensor("wq", [128, 2048], BF16, kind="ExternalInput").ap()
    wk_d = nc.dram_tensor("wk", [128, 2048], BF16, kind="ExternalInput").ap()
    wvt_d = nc.dram_tensor("wvt", [128, 2048], BF16, kind="ExternalInput").ap()
    wo_d = nc.dram_tensor("wo", [128, 2048], BF16, kind="ExternalInput").ap()
    tri_d = nc.dram_tensor("tri", [128, 128], BF16, kind="ExternalInput").ap()
    onesb_d = nc.dram_tensor("onesb", [128, 64], BF16,
                             kind="ExternalInput").ap()
    onesr_d = nc.dram_tensor("onesr", [1, 64], F32R, kind="ExternalInput").ap()
    out_d = nc.dram_tensor("out", [S, D], BF16, kind="ExternalOutput").ap()

    with tile.TileContext(nc) as tc, ExitStack() as ctx:
        consts = ctx.enter_context(tc.tile_pool(name="consts", bufs=1))
        sb = ctx.enter_context(tc.tile_pool(name="sb", bufs=1))
        ps = ctx.enter_context(tc.tile_pool(name="ps", bufs=1, space="PSUM"))

        # ---- DMA program: weights on the ACT hwdge queue, x on sync ----
        wq_t = consts.tile([128, 2048], BF16, tag="wq")
        nc.scalar.dma_start(wq_t, wq_d)
        wk_t = consts.tile([128, 2048], BF16, tag="wk")
        nc.scalar.dma_start(wk_t, wk_d)
        wvt_t = consts.tile([128, 2048], BF16, tag="wvt")
        nc.scalar.dma_start(wvt_t, wvt_d)
        wo_t = consts.tile([128, 2048], BF16, tag="wo")
        nc.scalar.dma_start(wo_t, wo_d)
        xtq = [[None] * 4 for _ in range(8)]
        for tc2 in range(4):
            for k in range(8):
                xk = sb.tile([128, 512], BF16, tag="xtq", bufs=32,
                             name=f"xtq{k}_{tc2}")
                nc.sync.dma_start(
                    xk, xt_d[128 * k:128 * (k + 1), 512 * tc2:512 * (tc2 + 1)])
                xtq[k][tc2] = xk
        tri_t = consts.tile([128, 128], BF16, tag="tri")
        nc.gpsimd.dma_start(tri_t, tri_d)
        onesb_t = consts.tile([128, 64], BF16, tag="onesb")
        nc.gpsimd.dma_start(onesb_t, onesb_d)
        # ones row at partition 64 (same base partition as the den row)
        onesr_t = consts.tile([65, 64], F32R, tag="onesr")
        nc.gpsimd.dma_start(onesr_t[64:65, :], onesr_d)

        qt = sb.tile([128, 2, 2048], BF16, tag="qt")
        kt = sb.tile([128, 2, 2048], BF16, tag="kt")
        vg = sb.tile([128, 16, NH, 65], BF16, tag="vg")
        nc.gpsimd.dma_start(vg[:, :, :, 64:65], onesb_d[:, 0:64])

        def qk_chain(w_t, dst, gg, tc2, on_act=False):
            pp = ps.tile([128, 512], F32, tag="mm", bufs=4)
            for k in range(8):
                nc.tensor.matmul(
                    pp,
                    lhsT=w_t[:, 1024 * gg + 128 * k:1024 * gg + 128 * (k + 1)],
                    rhs=xtq[k][tc2],
                    start=(k == 0), stop=(k == 7), skip_group_check=True)
            d = dst[:, gg, 512 * tc2:512 * (tc2 + 1)]
            if on_act:
                nc.scalar.activation(d, pp, COPY)
            else:
                nc.vector.tensor_copy(d, pp)

        def v_chain(j):
            vp = ps.tile([128, 256], F32, tag="mm", bufs=4)
            for k in range(8):
                nc.tensor.matmul(
                    vp,
                    lhsT=xtq[k][j // 4][:, 128 * (j % 4):128 * (j % 4 + 1)],
                    rhs=wvt_t[:, 256 * k:256 * (k + 1)],
                    start=(k == 0), stop=(k == 7), skip_group_check=True)
            nc.vector.tensor_copy(vg[:, j, :, 0:64], vp)

        # ---- deferred normalize (part1) / O projection (part2) ----
        avps = {}
        raw_of = {}
        pair_of = {}

        def part1_raw(qc, on_act=False):
            raws = []
            for h in range(NH):
                raw = sb.tile([65, 512], F32R, tag="raw", bufs=8,
                              name=f"raw{qc}_{h}")
                if on_act:
                    nc.scalar.activation(raw, avps[h][0:65, :], COPY)
                else:
                    nc.vector.tensor_copy(raw, avps[h][0:65, :])
                raws.append(raw)
            raw_of[qc] = raws

        def part1_norm(qc):
            raws = raw_of[qc]
            pairs = [sb.tile([128, 512], BF16, tag="pair", bufs=4,
                             name=f"pair{qc}_{p}") for p in range(2)]
            for h in range(NH):
                bc = ps.tile([64, 512], F32, tag="mm", bufs=4,
                             name=f"bc{qc}_{h}")
                nc.tensor.matmul(bc, lhsT=onesr_t[64:65, 0:64],
                                 rhs=raws[h][64:65, :],
                                 start=True, stop=True,
                                 skip_group_check=True)
                rcp = sb.tile([64, 512], F32, tag="rcp", bufs=4,
                              name=f"rcp{qc}_{h}")
                nc.vector.reciprocal_approx_fast(rcp, bc)
                if h % 2 == 0:
                    nc.vector.tensor_mul(pairs[h // 2][0:64, :],
                                         raws[h][0:64, :], rcp)
                else:
                    tmp = sb.tile([64, 512], BF16, tag="tmp", bufs=2,
                                  name=f"tmp{qc}_{h}")
                    nc.vector.tensor_mul(tmp, raws[h][0:64, :], rcp)
                    nc.sync.dma_start(pairs[h // 2][64:128, :], tmp)
            pair_of[qc] = pairs

        def part2_unit(qc, tt):
            pairs = pair_of[qc]
            row0 = 512 * qc + 128 * tt
            for chv in range(2):
                op = ps.tile([128, 512], F32, tag="mm", bufs=4,
                             name=f"op{qc}_{tt}_{chv}")
                nc.tensor.matmul(
                    op, lhsT=pairs[0][:, 128 * tt:128 * (tt + 1)],
                    rhs=wo_t[:, 512 * chv:512 * (chv + 1)],
                    start=True, stop=False, skip_group_check=True)
                nc.tensor.matmul(
                    op, lhsT=pairs[1][:, 128 * tt:128 * (tt + 1)],
                    rhs=wo_t[:, 1024 + 512 * chv:1024 + 512 * (chv + 1)],
                    start=False, stop=True, skip_group_check=True)
                ost = sb.tile([128, 512], BF16, tag="ost", bufs=4,
                              name=f"ost{qc}_{tt}_{chv}")
                nc.vector.tensor_copy(ost, op)
                nc.sync.dma_start(
                    out_d[row0:row0 + 128, 512 * chv:512 * (chv + 1)], ost)

        # ---- attention ----
        def attention(qc, fillers):
            njt = 4 * qc + 4
            for h in range(NH):
                avps[h] = ps.tile([128, 512], F32, tag=f"av{h}", bufs=1,
                                  name=f"avps{qc}_{h}")
            nfl = len(fillers)
            pend = []

            def do_av(j, ets):
                vs = max(0, 128 * (j - 4 * qc))
                for h in range(NH):
                    nc.tensor.matmul(
                        avps[h][0:65, vs:512],
                        lhsT=vg[:, j, h, 0:65],
                        rhs=ets[h][:, vs:512],
                        start=(j == 0), stop=(j == njt - 1),
                        skip_group_check=True)

            for j in range(njt):
                vs = max(0, 128 * (j - 4 * qc))
                ets = []
                for h in range(NH):
                    hp = 64 * (h % 2)
                    gg = h // 2
                    sc = ps.tile([128, 512], F32, tag="mm", bufs=4)
                    nc.tensor.matmul(
                        sc[:, vs:512],
                        lhsT=kt[hp:hp + 64, gg, 128 * j:128 * (j + 1)],
                        rhs=qt[hp:hp + 64, gg, 512 * qc + vs:512 * (qc + 1)],
                        start=True, stop=True)
                    et = sb.tile([128, 512], BF16, tag=f"et{h}", bufs=3)
                    nc.scalar.activation(et[:, vs:512], sc[:, vs:512],
                                         EXP, scale=SCALE)
                    if j >= 4 * qc:
                        nc.vector.tensor_mul(et[:, vs:vs + 128],
                                             et[:, vs:vs + 128], tri_t)
                    ets.append(et)
                pend.append((j, ets))
                if len(pend) > 1:   # lag-1 AV pipeline
                    do_av(*pend.pop(0))
                k0 = nfl * j // njt
                k1 = nfl * (j + 1) // njt
                for k in range(k0, k1):
                    fillers[k]()
            for args in pend:
                do_av(*args)
            part1_raw(qc, on_act=(qc == 3))

        # ---- schedule ----
        # pre-phase: just enough projections for attention(0)
        qk_chain(wq_t, qt, 0, 0, on_act=True)
        qk_chain(wq_t, qt, 1, 0, on_act=True)
        qk_chain(wk_t, kt, 0, 0, on_act=True)
        qk_chain(wk_t, kt, 1, 0, on_act=True)
        for j in range(4):
            v_chain(j)

        attention(0, [
            lambda: qk_chain(wk_t, kt, 0, 1),
            lambda: qk_chain(wk_t, kt, 1, 1),
            lambda: qk_chain(wq_t, qt, 0, 1),
            lambda: qk_chain(wq_t, qt, 1, 1),
            lambda: v_chain(4),
            lambda: v_chain(5),
        ])
        attention(1, [
            lambda: v_chain(6),
            lambda: v_chain(7),
            lambda: qk_chain(wk_t, kt, 0, 2),
            lambda: qk_chain(wk_t, kt, 1, 2),
            lambda: part1_norm(0),
            lambda: qk_chain(wq_t, qt, 0, 2),
            lambda: qk_chain(wq_t, qt, 1, 2),
            lambda: v_chain(8),
            lambda: part2_unit(0, 0),
            lambda: part2_unit(0, 1),
            lambda: part2_unit(0, 2),
        ])
        attention(2, [
            lambda: v_chain(9),
            lambda: v_chain(10),
            lambda: v_chain(11),
            lambda: qk_chain(wk_t, kt, 0, 3),
            lambda: qk_chain(wk_t, kt, 1, 3),
            lambda: part2_unit(0, 3),
            lambda: part1_norm(1),
            lambda: qk_chain(wq_t, qt, 0, 3),
            lambda: qk_chain(wq_t, qt, 1, 3),
            lambda: part2_unit(1, 0),
            lambda: part2_unit(1, 1),
            lambda: v_chain(12),
        ])
        attention(3, [
            lambda: v_chain(13),
            lambda: v_chain(14),
            lambda: v_chain(15),
            lambda: part2_unit(1, 2),
            lambda: part2_unit(1, 3),
            lambda: part1_norm(2),
            lambda: part2_unit(2, 0),
            lambda: part2_unit(2, 1),
            lambda: part2_unit(2, 2),
            lambda: part2_unit(2, 3),
        ])
        # tail
        part1_norm(3)
        for tt in range(4):
            part2_unit(3, tt)
    nc.compile()
    return nc


def _get_built():
    global _BUILT
    if _BUILT is None:
        _BUILT = _build()
    return _BUILT


def _host_inputs(x, q_proj, k_proj, v_proj, o_proj):
    bf = ml_dtypes.bfloat16
    xt = [np.ascontiguousarray(x[b].T.astype(bf)) for b in range(B)]
    tri = np.triu(np.ones((128, 128), dtype=np.float32)).astype(bf)
    onesb = np.ones((128, 64), dtype=np.float32).astype(bf)
    onesr = np.ones((1, 64), dtype=np.float32)

    def wqk(w, g):
        # [fp, 1024*gg + 128*k + m] = w[256g+128gg+m, 128k+fp]
        a = w[256 * g:256 * (g + 1)].reshape(2, 128, 8, 128)
        return np.ascontiguousarray(
            a.transpose(3, 0, 2, 1).reshape(128, 2048).astype(bf))

    def wvt(w, g):
        # [fp, 256*k + vd] = w[256g+vd, 128k+fp]
        a = w[256 * g:256 * (g + 1)].reshape(256, 8, 128)
        return np.ascontiguousarray(
            a.transpose(2, 1, 0).reshape(128, 2048).astype(bf))

    def wo_s(w, g):
        # [dp, 1024*dd + o] = o_proj[o, 256g+128dd+dp]
        a = w[:, 256 * g:256 * (g + 1)].reshape(1024, 2, 128)
        return np.ascontiguousarray(
            a.transpose(2, 1, 0).reshape(128, 2048).astype(bf))

    in_maps = []
    for c in range(NCORES):
        b, g = c // 4, c % 4
        in_maps.append(dict(
            xt=xt[b], wq=wqk(q_proj, g), wk=wqk(k_proj, g),
            wvt=wvt(v_proj, g), wo=wo_s(o_proj, g), tri=tri,
            onesb=onesb, onesr=onesr))
    return in_maps


def kernel(**inputs):
    x = np.asarray(inputs["x"], dtype=np.float32)
    q_proj = np.asarray(inputs["q_proj"], dtype=np.float32)
    k_proj = np.asarray(inputs["k_proj"], dtype=np.float32)
    v_proj = np.asarray(inputs["v_proj"], dtype=np.float32)
    o_proj = np.asarray(inputs["o_proj"], dtype=np.float32)

    in_maps = _host_inputs(x, q_proj, k_proj, v_proj, o_proj)
    nc = _get_built()
    global LAST_RESULTS
    LAST_RESULTS = run_bass_kernel_spmd(
        nc, in_maps, core_ids=list(range(NCORES)),
        trace=bool(os.environ.get("KERNEL_TRACE")))
    out = np.zeros((B, S, D), dtype=np.float32)
    for c in range(NCORES):
        out[c // 4] += np.asarray(
            LAST_RESULTS.results[c]["out"]).astype(np.float32)
    return out


# revision 14
# speedup vs baseline: 1.5132x; 1.0499x over previous
"""Multi-head self-attention (B=2, S=2048, D=1024, H=16, causal) on 8 trn2 cores.

Sharding: core c = (batch b=c//4, head-group g=c%4 of 4 heads = dims
256g:256g+256). Column-parallel QKV, row-parallel O; each core returns a
partial [2048, 1024] output for its batch; host sums 4 partials per batch.

Per-core kernel (all matmul inputs bf16):
  - projections from xt [1024, 2048] bf16: QT/KT [128, 2, 2048] bf16;
    V computed directly in token-major layout vg[128, 16, 4, 65] with a
    ones column per head so AV also produces the softmax denominator.
  - attention in transposed-score layout: scoresT[k, q] = K @ Q^T tiles,
    exp on ACT (scale 1/8 fused) to bf16, causal staircase skips invalid
    columns, triangular mask on diagonal blocks only. Lag-1 AV pipeline.
  - software pipeline: only the first token-chunk of Q/K and V tiles 0-3
    are projected up front; the remaining projection chains run as PE
    fillers inside the attention j-loops (which are paced by the exp
    stream on ACT), as do the deferred normalize + O-projection units.
  - normalize: raw AV copy to SBUF (frees PSUM), denominator broadcast
    via f32r ones outer-product matmul, reciprocal_approx_fast on DVE
    (no Ln -> no ACT table switches), multiply to bf16 pairs; odd heads
    DMA-shifted to partitions 64:128 so O contracts 128 dims per pair.
  - O projection: 2 chained matmuls per 512-col chunk; per-chunk drain
    and output DMA.
"""

import os
import numpy as np
from contextlib import ExitStack

import ml_dtypes

import concourse.bass as bass
import concourse.tile as tile
from concourse import bacc, mybir
from concourse.bass_utils import run_bass_kernel_spmd

F32R = mybir.dt.float32r
F32 = mybir.dt.float32
BF16 = mybir.dt.bfloat16
EXP = mybir.ActivationFunctionType.Exp
COPY = mybir.ActivationFunctionType.Copy

B, S, D = 2, 2048, 1024
NCORES = 8
SCALE = 0.125         # 1/sqrt(64)
NH = 4                # heads per core

_BUILT = None
LAST_RESULTS = None


def _build():
    nc = bacc.Bacc("TRN2", target_bir_lowering=False, debug=False,
                   num_devices=NCORES)
    # xt packed [p, tc, k, c]: xt[p, tc, k, c] = x[b][512tc+c, 128k+p]
    xt_d = nc.dram_tensor("xt", [128, 4, 8, 512], BF16,
                          kind="ExternalInput").ap()
    wq_d = nc.dram_tensor("wq", [128, 2048], BF16, kind="ExternalInput").ap()
    wk_d = nc.dram_tensor("wk", [128, 2048], BF16, kind="ExternalInput").ap()
    wvt_d = nc.dram_tensor("wvt", [128, 2048], BF16, kind="ExternalInput").ap()
    wo_d = nc.dram_tensor("wo", [128, 2048], BF16, kind="ExternalInput").ap()
    tri_d = nc.dram_tensor("tri", [128, 128], BF16, kind="ExternalInput").ap()
    onesb_d = nc.dram_tensor("onesb", [128, 64], BF16,
                             kind="ExternalInput").ap()
    onesr_d = nc.dram_tensor("onesr", [1, 64], F32R, kind="ExternalInput").ap()
    out_d = nc.dram_tensor("out", [S, D], BF16, kind="ExternalOutput").ap()

    with tile.TileContext(nc) as tc, ExitStack() as ctx:
        consts = ctx.enter_context(tc.tile_pool(name="consts", bufs=1))
        sb = ctx.enter_context(tc.tile_pool(name="sb", bufs=1))
        ps = ctx.enter_context(tc.tile_pool(name="ps", bufs=1, space="PSUM"))

        # ---- DMA program: weights on the ACT hwdge queue, x on sync ----
        wq_t = consts.tile([128, 2048], BF16, tag="wq")
        nc.scalar.dma_start(wq_t, wq_d)
        wk_t = consts.tile([128, 2048], BF16, tag="wk")
        nc.scalar.dma_start(wk_t, wk_d)
        wvt_t = consts.tile([128, 2048], BF16, tag="wvt")
        nc.scalar.dma_start(wvt_t, wvt_d)
        wo_t = consts.tile([128, 2048], BF16, tag="wo")
        nc.scalar.dma_start(wo_t, wo_d)
        # x: one big contiguous DMA per token-chunk (tc0 split for latency)
        xts = []
        for tc2 in range(4):
            xk = sb.tile([128, 8, 512], BF16, tag="xts", bufs=4,
                         name=f"xts{tc2}")
            if tc2 == 0:
                nc.sync.dma_start(xk[:, 0:4, :], xt_d[:, 0, 0:4, :])
                nc.sync.dma_start(xk[:, 4:8, :], xt_d[:, 0, 4:8, :])
            else:
                nc.sync.dma_start(xk, xt_d[:, tc2, :, :])
            xts.append(xk)
        tri_t = consts.tile([128, 128], BF16, tag="tri")
        nc.gpsimd.dma_start(tri_t, tri_d)
        onesb_t = consts.tile([128, 64], BF16, tag="onesb")
        nc.gpsimd.dma_start(onesb_t, onesb_d)
        # ones row at partition 64 (same base partition as the den row)
        onesr_t = consts.tile([65, 64], F32R, tag="onesr")
        nc.gpsimd.dma_start(onesr_t[64:65, :], onesr_d)

        qt = sb.tile([128, 2, 2048], BF16, tag="qt")
        kt = sb.tile([128, 2, 2048], BF16, tag="kt")
        vg = sb.tile([128, 16, NH, 65], BF16, tag="vg")
        # strided ones fill on the Pool engine (a strided DMA here would
        # degenerate to 2-byte SWDGE packets)
        nc.gpsimd.tensor_copy(vg[:, :, :, 64:65], onesb_t[:, 0:64])

        def qk_chain(w_t, dst, gg, tc2):
            pp = ps.tile([128, 512], F32, tag="mm", bufs=4)
            for k in range(8):
                nc.tensor.matmul(
                    pp,
                    lhsT=w_t[:, 1024 * gg + 128 * k:1024 * gg + 128 * (k + 1)],
                    rhs=xts[tc2][:, k, :],
                    start=(k == 0), stop=(k == 7), skip_group_check=True)
            nc.vector.tensor_copy(dst[:, gg, 512 * tc2:512 * (tc2 + 1)], pp)

        def v_chain(j):
            vp = ps.tile([128, 256], F32, tag="mm", bufs=4)
            for k in range(8):
                nc.tensor.matmul(
                    vp,
                    lhsT=xts[j // 4][:, k, 128 * (j % 4):128 * (j % 4 + 1)],
                    rhs=wvt_t[:, 256 * k:256 * (k + 1)],
                    start=(k == 0), stop=(k == 7), skip_group_check=True)
            nc.vector.tensor_copy(vg[:, j, :, 0:64], vp)

        # ---- deferred normalize (part1) / O projection (part2) ----
        avps = {}
        raw_of = {}
        pair_of = {}

        def part1_raw(qc, on_act=False):
            raws = []
            for h in range(NH):
                raw = sb.tile([65, 512], F32R, tag="raw", bufs=8,
                              name=f"raw{qc}_{h}")
                if on_act:
                    nc.scalar.activation(raw, avps[h][0:65, :], COPY)
                else:
                    nc.vector.tensor_copy(raw, avps[h][0:65, :])
                raws.append(raw)
            raw_of[qc] = raws

        def part1_norm(qc):
            raws = raw_of[qc]
            pairs = [sb.tile([128, 512], BF16, tag="pair", bufs=4,
                             name=f"pair{qc}_{p}") for p in range(2)]
            for h in range(NH):
                bc = ps.tile([64, 512], F32, tag="mm", bufs=4,
                             name=f"bc{qc}_{h}")
                nc.tensor.matmul(bc, lhsT=onesr_t[64:65, 0:64],
                                 rhs=raws[h][64:65, :],
                                 start=True, stop=True,
                                 skip_group_check=True)
                rcp = sb.tile([64, 512], F32, tag="rcp", bufs=4,
                              name=f"rcp{qc}_{h}")
                nc.vector.reciprocal_approx_fast(rcp, bc)
                if h % 2 == 0:
                    nc.vector.tensor_mul(pairs[h // 2][0:64, :],
                                         raws[h][0:64, :], rcp)
                else:
                    tmp = sb.tile([64, 512], BF16, tag="tmp", bufs=2,
                                  name=f"tmp{qc}_{h}")
                    nc.vector.tensor_mul(tmp, raws[h][0:64, :], rcp)
                    nc.sync.dma_start(pairs[h // 2][64:128, :], tmp)
            pair_of[qc] = pairs

        def part2_unit(qc, tt):
            pairs = pair_of[qc]
            row0 = 512 * qc + 128 * tt
            for chv in range(2):
                op = ps.tile([128, 512], F32, tag="mm", bufs=4,
                             name=f"op{qc}_{tt}_{chv}")
                nc.tensor.matmul(
                    op, lhsT=pairs[0][:, 128 * tt:128 * (tt + 1)],
                    rhs=wo_t[:, 512 * chv:512 * (chv + 1)],
                    start=True, stop=False, skip_group_check=True)
                nc.tensor.matmul(
                    op, lhsT=pairs[1][:, 128 * tt:128 * (tt + 1)],
                    rhs=wo_t[:, 1024 + 512 * chv:1024 + 512 * (chv + 1)],
                    start=False, stop=True, skip_group_check=True)
                ost = sb.tile([128, 512], BF16, tag="ost", bufs=4,
                              name=f"ost{qc}_{tt}_{chv}")
                nc.vector.tensor_copy(ost, op)
                nc.sync.dma_start(
                    out_d[row0:row0 + 128, 512 * chv:512 * (chv + 1)], ost)

        # ---- attention ----
        def attention(qc, fillers):
            njt = 4 * qc + 4
            for h in range(NH):
                avps[h] = ps.tile([128, 512], F32, tag=f"av{h}", bufs=1,
                                  name=f"avps{qc}_{h}")
            nfl = len(fillers)
            pend = []

            def do_av(j, ets):
                vs = max(0, 128 * (j - 4 * qc))
                for h in range(NH):
                    nc.tensor.matmul(
                        avps[h][0:65, vs:512],
                        lhsT=vg[:, j, h, 0:65],
                        rhs=ets[h][:, vs:512],
                        start=(j == 0), stop=(j == njt - 1),
                        skip_group_check=True)

            for j in range(njt):
                vs = max(0, 128 * (j - 4 * qc))
                ets = []
                for h in range(NH):
                    hp = 64 * (h % 2)
                    gg = h // 2
                    sc = ps.tile([128, 512], F32, tag="mm", bufs=4)
                    nc.tensor.matmul(
                        sc[:, vs:512],
                        lhsT=kt[hp:hp + 64, gg, 128 * j:128 * (j + 1)],
                        rhs=qt[hp:hp + 64, gg, 512 * qc + vs:512 * (qc + 1)],
                        start=True, stop=True)
                    et = sb.tile([128, 512], BF16, tag=f"et{h}", bufs=3)
                    nc.scalar.activation(et[:, vs:512], sc[:, vs:512],
                                         EXP, scale=SCALE)
                    if j >= 4 * qc:
                        nc.vector.tensor_mul(et[:, vs:vs + 128],
                                             et[:, vs:vs + 128], tri_t)
                    ets.append(et)
                pend.append((j, ets))
                if len(pend) > 1:   # lag-1 AV pipeline
                    do_av(*pend.pop(0))
                k0 = nfl * j // njt
                k1 = nfl * (j + 1) // njt
                for k in range(k0, k1):
                    fillers[k]()
            for args in pend:
                do_av(*args)
            part1_raw(qc, on_act=(qc == 3))

        def part2_tail():
            """qc=3 O projection: all pairA halves batched before pairB so
            the PE streams while the odd heads' normalize finishes."""
            pairs = pair_of[3]
            for half in range(2):
                ops = []
                for tt in (2 * half, 2 * half + 1):
                    for chv in range(2):
                        op = ps.tile([128, 512], F32, tag="mm", bufs=4,
                                     name=f"opt{tt}_{chv}")
                        nc.tensor.matmul(
                            op, lhsT=pairs[0][:, 128 * tt:128 * (tt + 1)],
                            rhs=wo_t[:, 512 * chv:512 * (chv + 1)],
                            start=True, stop=False, skip_group_check=True)
                        ops.append((tt, chv, op))
                for n, (tt, chv, op) in enumerate(ops):
                    nc.tensor.matmul(
                        op, lhsT=pairs[1][:, 128 * tt:128 * (tt + 1)],
                        rhs=wo_t[:, 1024 + 512 * chv:1024 + 512 * (chv + 1)],
                        start=False, stop=True, skip_group_check=True)
                    ost = sb.tile([128, 512], BF16, tag="ost", bufs=4,
                                  name=f"ostt{tt}_{chv}")
                    if n % 2 == 0:
                        nc.scalar.activation(ost, op, COPY)
                    else:
                        nc.vector.tensor_copy(ost, op)
                    row0 = 512 * 3 + 128 * tt
                    nc.sync.dma_start(
                        out_d[row0:row0 + 128,
                              512 * chv:512 * (chv + 1)], ost)

        # ---- schedule ----
        # pre-phase: just enough projections for attention(0)
        qk_chain(wq_t, qt, 0, 0)
        qk_chain(wq_t, qt, 1, 0)
        qk_chain(wk_t, kt, 0, 0)
        qk_chain(wk_t, kt, 1, 0)
        for j in range(4):
            v_chain(j)

        attention(0, [
            lambda: qk_chain(wk_t, kt, 0, 1),
            lambda: qk_chain(wk_t, kt, 1, 1),
            lambda: qk_chain(wq_t, qt, 0, 1),
            lambda: qk_chain(wq_t, qt, 1, 1),
            lambda: v_chain(4),
            lambda: v_chain(5),
        ])
        attention(1, [
            lambda: v_chain(6),
            lambda: v_chain(7),
            lambda: qk_chain(wk_t, kt, 0, 2),
            lambda: qk_chain(wk_t, kt, 1, 2),
            lambda: part1_norm(0),
            lambda: qk_chain(wq_t, qt, 0, 2),
            lambda: qk_chain(wq_t, qt, 1, 2),
            lambda: v_chain(8),
            lambda: part2_unit(0, 0),
            lambda: part2_unit(0, 1),
            lambda: part2_unit(0, 2),
        ])
        attention(2, [
            lambda: v_chain(9),
            lambda: v_chain(10),
            lambda: v_chain(11),
            lambda: qk_chain(wk_t, kt, 0, 3),
            lambda: qk_chain(wk_t, kt, 1, 3),
            lambda: part2_unit(0, 3),
            lambda: part1_norm(1),
            lambda: qk_chain(wq_t, qt, 0, 3),
            lambda: qk_chain(wq_t, qt, 1, 3),
            lambda: part2_unit(1, 0),
            lambda: part2_unit(1, 1),
            lambda: v_chain(12),
        ])
        attention(3, [
            lambda: v_chain(13),
            lambda: v_chain(14),
            lambda: v_chain(15),
            lambda: part2_unit(1, 2),
            lambda: part2_unit(1, 3),
            lambda: part1_norm(2),
            lambda: part2_unit(2, 0),
            lambda: part2_unit(2, 1),
            lambda: part2_unit(2, 2),
            lambda: part2_unit(2, 3),
        ])
        # tail
        part1_norm(3)
        part2_tail()
    nc.compile()
    return nc


def _get_built():
    global _BUILT
    if _BUILT is None:
        _BUILT = _build()
    return _BUILT


def _host_inputs(x, q_proj, k_proj, v_proj, o_proj):
    bf = ml_dtypes.bfloat16
    # [p, tc, k, c] = x[b][512tc+c, 128k+p]
    xt = [np.ascontiguousarray(
        x[b].T.reshape(8, 128, 4, 512).transpose(1, 2, 0, 3).astype(bf))
        for b in range(B)]
    tri = np.triu(np.ones((128, 128), dtype=np.float32)).astype(bf)
    onesb = np.ones((128, 64), dtype=np.float32).astype(bf)
    onesr = np.ones((1, 64), dtype=np.float32)

    def wqk(w, g):
        # [fp, 1024*gg + 128*k + m] = w[256g+128gg+m, 128k+fp]
        a = w[256 * g:256 * (g + 1)].reshape(2, 128, 8, 128)
        return np.ascontiguousarray(
            a.transpose(3, 0, 2, 1).reshape(128, 2048).astype(bf))

    def wvt(w, g):
        # [fp, 256*k + vd] = w[256g+vd, 128k+fp]
        a = w[256 * g:256 * (g + 1)].reshape(256, 8, 128)
        return np.ascontiguousarray(
            a.transpose(2, 1, 0).reshape(128, 2048).astype(bf))

    def wo_s(w, g):
        # [dp, 1024*dd + o] = o_proj[o, 256g+128dd+dp]
        a = w[:, 256 * g:256 * (g + 1)].reshape(1024, 2, 128)
        return np.ascontiguousarray(
            a.transpose(2, 1, 0).reshape(128, 2048).astype(bf))

    in_maps = []
    for c in range(NCORES):
        b, g = c // 4, c % 4
        in_maps.append(dict(
            xt=xt[b], wq=wqk(q_proj, g), wk=wqk(k_proj, g),
            wvt=wvt(v_proj, g), wo=wo_s(o_proj, g), tri=tri,
            onesb=onesb, onesr=onesr))
    return in_maps


def kernel(**inputs):
    x = np.asarray(inputs["x"], dtype=np.float32)
    q_proj = np.asarray(inputs["q_proj"], dtype=np.float32)
    k_proj = np.asarray(inputs["k_proj"], dtype=np.float32)
    v_proj = np.asarray(inputs["v_proj"], dtype=np.float32)
    o_proj = np.asarray(inputs["o_proj"], dtype=np.float32)

    in_maps = _host_inputs(x, q_proj, k_proj, v_proj, o_proj)
    nc = _get_built()
    global LAST_RESULTS
    LAST_RESULTS = run_bass_kernel_spmd(
        nc, in_maps, core_ids=list(range(NCORES)),
        trace=bool(os.environ.get("KERNEL_TRACE")))
    out = np.zeros((B, S, D), dtype=np.float32)
    for c in range(NCORES):
        out[c // 4] += np.asarray(
            LAST_RESULTS.results[c]["out"]).astype(np.float32)
    return out


# revision 20
# speedup vs baseline: 1.5927x; 1.0525x over previous
"""Multi-head self-attention (B=2, S=2048, D=1024, H=16, causal) on 8 trn2 cores.

Sharding: core c = (batch b=c//4, head-group g=c%4 of 4 heads = dims
256g:256g+256). Column-parallel QKV, row-parallel O; each core returns a
partial [2048, 1024] output for its batch; host sums 4 partials per batch.

Per-core kernel (all matmul inputs bf16):
  - projections from xt [1024, 2048] bf16: QT/KT [128, 2, 2048] bf16;
    V computed directly in token-major layout vg[128, 16, 4, 65] with a
    ones column per head so AV also produces the softmax denominator.
  - attention in transposed-score layout: scoresT[k, q] = K @ Q^T tiles,
    exp on ACT (scale 1/8 fused) to bf16, causal staircase skips invalid
    columns, triangular mask on diagonal blocks only. Lag-1 AV pipeline.
  - software pipeline: only the first token-chunk of Q/K and V tiles 0-3
    are projected up front; the remaining projection chains run as PE
    fillers inside the attention j-loops (which are paced by the exp
    stream on ACT), as do the deferred normalize + O-projection units.
  - normalize: raw AV copy to SBUF (frees PSUM), denominator broadcast
    via f32r ones outer-product matmul, reciprocal_approx_fast on DVE
    (no Ln -> no ACT table switches), multiply to bf16 pairs; odd heads
    DMA-shifted to partitions 64:128 so O contracts 128 dims per pair.
  - O projection: 2 chained matmuls per 512-col chunk; per-chunk drain
    and output DMA.
"""

import os
import numpy as np
from contextlib import ExitStack

import ml_dtypes

import concourse.bass as bass
import concourse.tile as tile
from concourse import bacc, mybir
from concourse.bass_utils import run_bass_kernel_spmd

F32R = mybir.dt.float32r
F32 = mybir.dt.float32
BF16 = mybir.dt.bfloat16
EXP = mybir.ActivationFunctionType.Exp
COPY = mybir.ActivationFunctionType.Copy

B, S, D = 2, 2048, 1024
NCORES = 8
SCALE = 0.125         # 1/sqrt(64)
NH = 4                # heads per core

_BUILT = None
LAST_RESULTS = None


def _build():
    nc = bacc.Bacc("TRN2", target_bir_lowering=False, debug=False,
                   num_devices=NCORES)
    # xt packed [p, tc, k, c]: xt[p, tc, k, c] = x[b][512tc+c, 128k+p]
    xt_d = nc.dram_tensor("xt", [128, 4, 8, 512], BF16,
                          kind="ExternalInput").ap()
    wq_d = nc.dram_tensor("wq", [128, 2048], BF16, kind="ExternalInput").ap()
    wk_d = nc.dram_tensor("wk", [128, 2048], BF16, kind="ExternalInput").ap()
    wvt_d = nc.dram_tensor("wvt", [128, 2048], BF16, kind="ExternalInput").ap()
    wo_d = nc.dram_tensor("wo", [128, 2048], BF16, kind="ExternalInput").ap()
    tri_d = nc.dram_tensor("tri", [128, 128], BF16, kind="ExternalInput").ap()
    onesb_d = nc.dram_tensor("onesb", [128, 64], BF16,
                             kind="ExternalInput").ap()
    onesr_d = nc.dram_tensor("onesr", [1, 64], F32R, kind="ExternalInput").ap()
    out_d = nc.dram_tensor("out", [S, D], BF16, kind="ExternalOutput").ap()

    with tile.TileContext(nc) as tc, ExitStack() as ctx:
        consts = ctx.enter_context(tc.tile_pool(name="consts", bufs=1))
        sb = ctx.enter_context(tc.tile_pool(name="sb", bufs=1))
        ps = ctx.enter_context(tc.tile_pool(name="ps", bufs=1, space="PSUM"))

        # ---- DMA program: weights on the ACT hwdge queue, x on sync ----
        # (halves so the first projection chains start sooner)
        wq_t = consts.tile([128, 2048], BF16, tag="wq")
        nc.scalar.dma_start(wq_t[:, 0:1024], wq_d[:, 0:1024])
        nc.scalar.dma_start(wq_t[:, 1024:2048], wq_d[:, 1024:2048])
        wk_t = consts.tile([128, 2048], BF16, tag="wk")
        nc.scalar.dma_start(wk_t[:, 0:1024], wk_d[:, 0:1024])
        nc.scalar.dma_start(wk_t[:, 1024:2048], wk_d[:, 1024:2048])
        wvt_t = consts.tile([128, 2048], BF16, tag="wvt")
        nc.scalar.dma_start(wvt_t, wvt_d)
        wo_t = consts.tile([128, 2048], BF16, tag="wo")
        nc.scalar.dma_start(wo_t, wo_d)
        # x: one big contiguous DMA per token-chunk (tc0 split for latency)
        xts = []
        for tc2 in range(4):
            xk = sb.tile([128, 8, 512], BF16, tag="xts", bufs=4,
                         name=f"xts{tc2}")
            if tc2 == 0:
                nc.sync.dma_start(xk[:, 0:4, :], xt_d[:, 0, 0:4, :])
                nc.sync.dma_start(xk[:, 4:8, :], xt_d[:, 0, 4:8, :])
            else:
                nc.sync.dma_start(xk, xt_d[:, tc2, :, :])
            xts.append(xk)
        tri_t = consts.tile([128, 128], BF16, tag="tri")
        nc.gpsimd.dma_start(tri_t, tri_d)
        onesb_t = consts.tile([128, 64], BF16, tag="onesb")
        nc.gpsimd.dma_start(onesb_t, onesb_d)
        # ones row at partition 64 (same base partition as the den row)
        onesr_t = consts.tile([65, 64], F32R, tag="onesr")
        nc.gpsimd.dma_start(onesr_t[64:65, :], onesr_d)

        qt = sb.tile([128, 2, 2048], BF16, tag="qt")
        kt = sb.tile([128, 2, 2048], BF16, tag="kt")
        vg = sb.tile([128, 16, NH, 65], BF16, tag="vg")
        # strided ones fill on the Pool engine (a strided DMA here would
        # degenerate to 2-byte SWDGE packets)
        nc.gpsimd.tensor_copy(vg[:, :, :, 64:65], onesb_t[:, 0:64])

        def qk_chain(w_t, dst, gg, tc2):
            pp = ps.tile([128, 512], F32, tag="mm", bufs=2)
            for k in range(8):
                nc.tensor.matmul(
                    pp,
                    lhsT=w_t[:, 1024 * gg + 128 * k:1024 * gg + 128 * (k + 1)],
                    rhs=xts[tc2][:, k, :],
                    start=(k == 0), stop=(k == 7), skip_group_check=True)
            nc.vector.tensor_copy(dst[:, gg, 512 * tc2:512 * (tc2 + 1)], pp)

        def v_chain(j):
            vp = ps.tile([128, 256], F32, tag="mm", bufs=2)
            for k in range(8):
                nc.tensor.matmul(
                    vp,
                    lhsT=xts[j // 4][:, k, 128 * (j % 4):128 * (j % 4 + 1)],
                    rhs=wvt_t[:, 256 * k:256 * (k + 1)],
                    start=(k == 0), stop=(k == 7), skip_group_check=True)
            nc.vector.tensor_copy(vg[:, j, :, 0:64], vp)

        # ---- deferred normalize (part1) / O projection (part2) ----
        raw_of = {}
        pair_of = {}

        def part1_norm(qc):
            raws = raw_of[qc]
            pairs = [sb.tile([128, 512], BF16, tag="pair", bufs=4,
                             name=f"pair{qc}_{p}") for p in range(2)]
            for h in range(NH):
                bc = ps.tile([64, 512], F32, tag="mm", bufs=2,
                             name=f"bc{qc}_{h}")
                nc.tensor.matmul(bc, lhsT=onesr_t[64:65, 0:64],
                                 rhs=raws[h][64:65, :],
                                 start=True, stop=True,
                                 skip_group_check=True)
                rcp = sb.tile([64, 512], F32, tag="rcp", bufs=4,
                              name=f"rcp{qc}_{h}")
                nc.vector.reciprocal_approx_fast(rcp, bc)
                if h % 2 == 0:
                    nc.vector.tensor_mul(pairs[h // 2][0:64, :],
                                         raws[h][0:64, :], rcp)
                else:
                    tmp = sb.tile([64, 512], BF16, tag="tmp", bufs=2,
                                  name=f"tmp{qc}_{h}")
                    nc.vector.tensor_mul(tmp, raws[h][0:64, :], rcp)
                    nc.sync.dma_start(pairs[h // 2][64:128, :], tmp)
            pair_of[qc] = pairs

        def part2_unit(qc, tt):
            pairs = pair_of[qc]
            row0 = 512 * qc + 128 * tt
            for chv in range(2):
                op = ps.tile([128, 512], F32, tag="mm", bufs=2,
                             name=f"op{qc}_{tt}_{chv}")
                nc.tensor.matmul(
                    op, lhsT=pairs[0][:, 128 * tt:128 * (tt + 1)],
                    rhs=wo_t[:, 512 * chv:512 * (chv + 1)],
                    start=True, stop=False, skip_group_check=True)
                nc.tensor.matmul(
                    op, lhsT=pairs[1][:, 128 * tt:128 * (tt + 1)],
                    rhs=wo_t[:, 1024 + 512 * chv:1024 + 512 * (chv + 1)],
                    start=False, stop=True, skip_group_check=True)
                ost = sb.tile([128, 512], BF16, tag="ost", bufs=4,
                              name=f"ost{qc}_{tt}_{chv}")
                if chv == 0:
                    nc.vector.tensor_copy(ost, op)
                else:
                    nc.scalar.activation(ost, op, COPY)
                nc.sync.dma_start(
                    out_d[row0:row0 + 128, 512 * chv:512 * (chv + 1)], ost)

        # ---- attention: two sweeps of 2 heads; 1024-wide exp per j ----
        def attention(qc, fillers):
            njt = 4 * qc + 4
            nfl = len(fillers)
            for sweep in range(2):
                h0 = 2 * sweep
                avs = [ps.tile([128, 512], F32, tag="av", bufs=2,
                               name=f"avps{qc}_{sweep}_{hh}")
                       for hh in range(2)]
                pend = []

                def do_av(j, etp, avs=avs, h0=h0):
                    vs = max(0, 128 * (j - 4 * qc))
                    for hh in range(2):
                        nc.tensor.matmul(
                            avs[hh][0:65, vs:512],
                            lhsT=vg[:, j, h0 + hh, 0:65],
                            rhs=etp[:, hh, vs:512],
                            start=(j == 0), stop=(j == njt - 1),
                            skip_group_check=True)

                for j in range(njt):
                    vs = max(0, 128 * (j - 4 * qc))
                    scp = ps.tile([128, 2, 512], F32, tag="mm2", bufs=2)
                    for hh in range(2):
                        h = h0 + hh
                        hp = 64 * (h % 2)
                        gg = h // 2
                        nc.tensor.matmul(
                            scp[:, hh, vs:512],
                            lhsT=kt[hp:hp + 64, gg, 128 * j:128 * (j + 1)],
                            rhs=qt[hp:hp + 64, gg,
                                   512 * qc + vs:512 * (qc + 1)],
                            start=True, stop=True, skip_group_check=True)
                    etp = sb.tile([128, 2, 512], BF16, tag=f"et{sweep}",
                                  bufs=3)
                    nc.scalar.activation(etp[:, :, vs:512], scp[:, :, vs:512],
                                         EXP, scale=SCALE)
                    if j >= 4 * qc:
                        for hh in range(2):
                            nc.vector.tensor_mul(etp[:, hh, vs:vs + 128],
                                                 etp[:, hh, vs:vs + 128],
                                                 tri_t)
                    pend.append((j, etp))
                    if len(pend) > 1:   # lag-1 AV pipeline
                        do_av(*pend.pop(0))
                    slot = sweep * njt + j
                    k0 = nfl * slot // (2 * njt)
                    k1 = nfl * (slot + 1) // (2 * njt)
                    for k in range(k0, k1):
                        fillers[k]()
                for args in pend:
                    do_av(*args)
                # drain AV psum at the sweep boundary (frees the av banks)
                for hh in range(2):
                    raw = sb.tile([65, 512], F32R, tag="raw", bufs=8,
                                  name=f"raw{qc}_{h0 + hh}")
                    if qc == 3 and sweep == 1:
                        nc.scalar.activation(raw, avs[hh][0:65, :], COPY)
                    else:
                        nc.vector.tensor_copy(raw, avs[hh][0:65, :])
                    raw_of.setdefault(qc, []).append(raw)

        # ---- schedule ----
        # pre-phase: just enough projections for attention(0)
        qk_chain(wq_t, qt, 0, 0)
        qk_chain(wq_t, qt, 1, 0)
        qk_chain(wk_t, kt, 0, 0)
        qk_chain(wk_t, kt, 1, 0)
        for j in range(4):
            v_chain(j)

        attention(0, [
            lambda: qk_chain(wk_t, kt, 0, 1),
            lambda: qk_chain(wk_t, kt, 1, 1),
            lambda: qk_chain(wq_t, qt, 0, 1),
            lambda: qk_chain(wq_t, qt, 1, 1),
            lambda: v_chain(4),
            lambda: v_chain(5),
        ])
        attention(1, [
            lambda: v_chain(6),
            lambda: v_chain(7),
            lambda: qk_chain(wk_t, kt, 0, 2),
            lambda: qk_chain(wk_t, kt, 1, 2),
            lambda: part1_norm(0),
            lambda: qk_chain(wq_t, qt, 0, 2),
            lambda: qk_chain(wq_t, qt, 1, 2),
            lambda: v_chain(8),
            lambda: part2_unit(0, 0),
            lambda: part2_unit(0, 1),
            lambda: part2_unit(0, 2),
        ])
        attention(2, [
            lambda: v_chain(9),
            lambda: v_chain(10),
            lambda: v_chain(11),
            lambda: qk_chain(wk_t, kt, 0, 3),
            lambda: qk_chain(wk_t, kt, 1, 3),
            lambda: part2_unit(0, 3),
            lambda: part1_norm(1),
            lambda: qk_chain(wq_t, qt, 0, 3),
            lambda: qk_chain(wq_t, qt, 1, 3),
            lambda: part2_unit(1, 0),
            lambda: part2_unit(1, 1),
            lambda: v_chain(12),
        ])
        attention(3, [
            lambda: v_chain(13),
            lambda: v_chain(14),
            lambda: v_chain(15),
            lambda: part2_unit(1, 2),
            lambda: part2_unit(1, 3),
            lambda: part1_norm(2),
            lambda: part2_unit(2, 0),
            lambda: part2_unit(2, 1),
            lambda: part2_unit(2, 2),
            lambda: part2_unit(2, 3),
        ])
        # tail
        part1_norm(3)
        for tt in range(4):
            part2_unit(3, tt)
    nc.compile()
    return nc


def _get_built():
    global _BUILT
    if _BUILT is None:
        _BUILT = _build()
    return _BUILT


def _host_inputs(x, q_proj, k_proj, v_proj, o_proj):
    bf = ml_dtypes.bfloat16
    # [p, tc, k, c] = x[b][512tc+c, 128k+p]
    xt = [np.ascontiguousarray(
        x[b].T.reshape(8, 128, 4, 512).transpose(1, 2, 0, 3).astype(bf))
        for b in range(B)]
    tri = np.triu(np.ones((128, 128), dtype=np.float32)).astype(bf)
    onesb = np.ones((128, 64), dtype=np.float32).astype(bf)
    onesr = np.ones((1, 64), dtype=np.float32)

    def wqk(w, g):
        # [fp, 1024*gg + 128*k + m] = w[256g+128gg+m, 128k+fp]
        a = w[256 * g:256 * (g + 1)].reshape(2, 128, 8, 128)
        return np.ascontiguousarray(
            a.transpose(3, 0, 2, 1).reshape(128, 2048).astype(bf))

    def wvt(w, g):
        # [fp, 256*k + vd] = w[256g+vd, 128k+fp]
        a = w[256 * g:256 * (g + 1)].reshape(256, 8, 128)
        return np.ascontiguousarray(
            a.transpose(2, 1, 0).reshape(128, 2048).astype(bf))

    def wo_s(w, g):
        # [dp, 1024*dd + o] = o_proj[o, 256g+128dd+dp]
        a = w[:, 256 * g:256 * (g + 1)].reshape(1024, 2, 128)
        return np.ascontiguousarray(
            a.transpose(2, 1, 0).reshape(128, 2048).astype(bf))

    in_maps = []
    for c in range(NCORES):
        b, g = c // 4, c % 4
        in_maps.append(dict(
            xt=xt[b], wq=wqk(q_proj, g), wk=wqk(k_proj, g),
            wvt=wvt(v_proj, g), wo=wo_s(o_proj, g), tri=tri,
            onesb=onesb, onesr=onesr))
    return in_maps


def kernel(**inputs):
    x = np.asarray(inputs["x"], dtype=np.float32)
    q_proj = np.asarray(inputs["q_proj"], dtype=np.float32)
    k_proj = np.asarray(inputs["k_proj"], dtype=np.float32)
    v_proj = np.asarray(inputs["v_proj"], dtype=np.float32)
    o_proj = np.asarray(inputs["o_proj"], dtype=np.float32)

    in_maps = _host_inputs(x, q_proj, k_proj, v_proj, o_proj)
    nc = _get_built()
    global LAST_RESULTS
    LAST_RESULTS = run_bass_kernel_spmd(
        nc, in_maps, core_ids=list(range(NCORES)),
        trace=bool(os.environ.get("KERNEL_TRACE")))
    out = np.zeros((B, S, D), dtype=np.float32)
    for c in range(NCORES):
        out[c // 4] += np.asarray(
            LAST_RESULTS.results[c]["out"]).astype(np.float32)
    return out


# revision 25
# speedup vs baseline: 1.6230x; 1.0190x over previous
"""Multi-head self-attention (B=2, S=2048, D=1024, H=16, causal) on 8 trn2 cores.

Sharding: core c = (batch b=c//4, head-group g=c%4 of 4 heads = dims
256g:256g+256). Column-parallel QKV, row-parallel O; each core returns a
partial [2048, 1024] output for its batch; host sums 4 partials per batch.

Per-core kernel (all matmul inputs bf16):
  - projections from xt [1024, 2048] bf16: QT/KT [128, 2, 2048] bf16;
    V computed directly in token-major layout vg[128, 16, 4, 65] with a
    ones column per head so AV also produces the softmax denominator.
  - attention in transposed-score layout: scoresT[k, q] = K @ Q^T tiles,
    exp on ACT (scale 1/8 fused) to bf16, causal staircase skips invalid
    columns, triangular mask on diagonal blocks only. Lag-1 AV pipeline.
  - software pipeline: only the first token-chunk of Q/K and V tiles 0-3
    are projected up front; the remaining projection chains run as PE
    fillers inside the attention j-loops (which are paced by the exp
    stream on ACT), as do the deferred normalize + O-projection units.
  - normalize: raw AV copy to SBUF (frees PSUM), denominator broadcast
    via f32r ones outer-product matmul, reciprocal_approx_fast on DVE
    (no Ln -> no ACT table switches), multiply to bf16 pairs; odd heads
    DMA-shifted to partitions 64:128 so O contracts 128 dims per pair.
  - O projection: 2 chained matmuls per 512-col chunk; per-chunk drain
    and output DMA.
"""

import os
import numpy as np
from contextlib import ExitStack

import ml_dtypes

import concourse.bass as bass
import concourse.tile as tile
from concourse import bacc, mybir
from concourse.bass_utils import run_bass_kernel_spmd

F32R = mybir.dt.float32r
F32 = mybir.dt.float32
BF16 = mybir.dt.bfloat16
EXP = mybir.ActivationFunctionType.Exp
COPY = mybir.ActivationFunctionType.Copy

B, S, D = 2, 2048, 1024
NCORES = 8
SCALE = 0.125         # 1/sqrt(64)
NH = 4                # heads per core

_BUILT = None
LAST_RESULTS = None


def _build():
    nc = bacc.Bacc("TRN2", target_bir_lowering=False, debug=False,
                   num_devices=NCORES)
    # xt packed [p, tc, k, c]: xt[p, tc, k, c] = x[b][512tc+c, 128k+p]
    xt_d = nc.dram_tensor("xt", [128, 4, 8, 512], BF16,
                          kind="ExternalInput").ap()
    wq_d = nc.dram_tensor("wq", [128, 2048], BF16, kind="ExternalInput").ap()
    wk_d = nc.dram_tensor("wk", [128, 2048], BF16, kind="ExternalInput").ap()
    wvt_d = nc.dram_tensor("wvt", [128, 2048], BF16, kind="ExternalInput").ap()
    wo_d = nc.dram_tensor("wo", [128, 2048], BF16, kind="ExternalInput").ap()
    tri_d = nc.dram_tensor("tri", [128, 128], BF16, kind="ExternalInput").ap()
    onesb_d = nc.dram_tensor("onesb", [128, 64], BF16,
                             kind="ExternalInput").ap()
    onesr_d = nc.dram_tensor("onesr", [1, 64], F32R, kind="ExternalInput").ap()
    out_d = nc.dram_tensor("out", [S, D], BF16, kind="ExternalOutput").ap()

    with tile.TileContext(nc) as tc, ExitStack() as ctx:
        consts = ctx.enter_context(tc.tile_pool(name="consts", bufs=1))
        sb = ctx.enter_context(tc.tile_pool(name="sb", bufs=1))
        ps = ctx.enter_context(tc.tile_pool(name="ps", bufs=1, space="PSUM"))

        # ---- DMA program: weights on the ACT hwdge queue, x on sync ----
        # (halves so the first projection chains start sooner)
        wq_t = consts.tile([128, 2048], BF16, tag="wq")
        nc.scalar.dma_start(wq_t[:, 0:1024], wq_d[:, 0:1024])
        nc.scalar.dma_start(wq_t[:, 1024:2048], wq_d[:, 1024:2048])
        wk_t = consts.tile([128, 2048], BF16, tag="wk")
        nc.scalar.dma_start(wk_t[:, 0:1024], wk_d[:, 0:1024])
        nc.scalar.dma_start(wk_t[:, 1024:2048], wk_d[:, 1024:2048])
        wvt_t = consts.tile([128, 2048], BF16, tag="wvt")
        nc.scalar.dma_start(wvt_t, wvt_d)
        wo_t = consts.tile([128, 2048], BF16, tag="wo")
        nc.scalar.dma_start(wo_t, wo_d)
        # x: one big contiguous DMA per token-chunk (tc0 split for latency)
        xts = []
        for tc2 in range(4):
            xk = sb.tile([128, 8, 512], BF16, tag="xts", bufs=4,
                         name=f"xts{tc2}")
            if tc2 == 0:
                for kq in range(4):
                    nc.sync.dma_start(xk[:, 2 * kq:2 * kq + 2, :],
                                      xt_d[:, 0, 2 * kq:2 * kq + 2, :])
            else:
                nc.sync.dma_start(xk, xt_d[:, tc2, :, :])
            xts.append(xk)
        tri_t = consts.tile([128, 128], BF16, tag="tri")
        nc.gpsimd.dma_start(tri_t, tri_d)
        onesb_t = consts.tile([128, 64], BF16, tag="onesb")
        nc.gpsimd.dma_start(onesb_t, onesb_d)
        # ones row at partition 64 (same base partition as the den row)
        onesr_t = consts.tile([65, 64], F32R, tag="onesr")
        nc.gpsimd.dma_start(onesr_t[64:65, :], onesr_d)

        qt = sb.tile([128, 2, 2048], BF16, tag="qt")
        kt = sb.tile([128, 2, 2048], BF16, tag="kt")
        vg = sb.tile([128, 16, NH, 65], BF16, tag="vg")
        # strided ones fill on the Pool engine (a strided DMA here would
        # degenerate to 2-byte SWDGE packets)
        nc.gpsimd.tensor_copy(vg[:, :, :, 64:65], onesb_t[:, 0:64])

        def qk_chain(w_t, dst, gg, tc2):
            pp = ps.tile([128, 512], F32, tag="mm", bufs=2)
            for k in range(8):
                nc.tensor.matmul(
                    pp,
                    lhsT=w_t[:, 1024 * gg + 128 * k:1024 * gg + 128 * (k + 1)],
                    rhs=xts[tc2][:, k, :],
                    start=(k == 0), stop=(k == 7), skip_group_check=True)
            nc.vector.tensor_copy(dst[:, gg, 512 * tc2:512 * (tc2 + 1)], pp)

        def v_chain(j):
            vp = ps.tile([128, 256], F32, tag="mm", bufs=2)
            for k in range(8):
                nc.tensor.matmul(
                    vp,
                    lhsT=xts[j // 4][:, k, 128 * (j % 4):128 * (j % 4 + 1)],
                    rhs=wvt_t[:, 256 * k:256 * (k + 1)],
                    start=(k == 0), stop=(k == 7), skip_group_check=True)
            nc.vector.tensor_copy(vg[:, j, :, 0:64], vp)

        # ---- deferred normalize (part1) / O projection (part2) ----
        raw_of = {}
        pair_of = {}

        def part1_norm(qc):
            raws = raw_of[qc]
            pairs = [sb.tile([128, 512], BF16, tag="pair", bufs=4,
                             name=f"pair{qc}_{p}") for p in range(2)]
            for h in range(NH):
                bc = ps.tile([64, 512], F32, tag="mm", bufs=2,
                             name=f"bc{qc}_{h}")
                nc.tensor.matmul(bc, lhsT=onesr_t[64:65, 0:64],
                                 rhs=raws[h][64:65, :],
                                 start=True, stop=True,
                                 skip_group_check=True)
                rcp = sb.tile([64, 512], F32, tag="rcp", bufs=4,
                              name=f"rcp{qc}_{h}")
                nc.vector.reciprocal_approx_fast(rcp, bc)
                if h % 2 == 0:
                    nc.vector.tensor_mul(pairs[h // 2][0:64, :],
                                         raws[h][0:64, :], rcp)
                else:
                    tmp = sb.tile([64, 512], BF16, tag="tmp", bufs=2,
                                  name=f"tmp{qc}_{h}")
                    nc.vector.tensor_mul(tmp, raws[h][0:64, :], rcp)
                    nc.scalar.dma_start(pairs[h // 2][64:128, :], tmp)
            pair_of[qc] = pairs

        def part2_unit(qc, tt):
            pairs = pair_of[qc]
            row0 = 512 * qc + 128 * tt
            ost = sb.tile([128, 1024], BF16, tag="ost", bufs=3,
                          name=f"ost{qc}_{tt}")
            for chv in range(2):
                op = ps.tile([128, 512], F32, tag="mm", bufs=2,
                             name=f"op{qc}_{tt}_{chv}")
                nc.tensor.matmul(
                    op, lhsT=pairs[0][:, 128 * tt:128 * (tt + 1)],
                    rhs=wo_t[:, 512 * chv:512 * (chv + 1)],
                    start=True, stop=False, skip_group_check=True)
                nc.tensor.matmul(
                    op, lhsT=pairs[1][:, 128 * tt:128 * (tt + 1)],
                    rhs=wo_t[:, 1024 + 512 * chv:1024 + 512 * (chv + 1)],
                    start=False, stop=True, skip_group_check=True)
                if chv == 0:
                    nc.vector.tensor_copy(ost[:, 0:512], op)
                else:
                    nc.scalar.activation(ost[:, 512:1024], op, COPY)
            nc.sync.dma_start(out_d[row0:row0 + 128, :], ost)

        # ---- attention: two sweeps of 2 heads; 1024-wide exp per j ----
        def attention(qc, fillers):
            njt = 4 * qc + 4
            nfl = len(fillers)
            for sweep in range(2):
                h0 = 2 * sweep
                avs = [ps.tile([128, 512], F32, tag="av", bufs=2,
                               name=f"avps{qc}_{sweep}_{hh}")
                       for hh in range(2)]
                pend = []

                def do_av(j, etp, avs=avs, h0=h0):
                    vs = max(0, 128 * (j - 4 * qc))
                    for hh in range(2):
                        nc.tensor.matmul(
                            avs[hh][0:65, vs:512],
                            lhsT=vg[:, j, h0 + hh, 0:65],
                            rhs=etp[:, hh, vs:512],
                            start=(j == 0), stop=(j == njt - 1),
                            skip_group_check=True)

                for j in range(njt):
                    vs = max(0, 128 * (j - 4 * qc))
                    scp = ps.tile([128, 2, 512], F32, tag="mm2", bufs=2)
                    for hh in range(2):
                        h = h0 + hh
                        hp = 64 * (h % 2)
                        gg = h // 2
                        nc.tensor.matmul(
                            scp[:, hh, vs:512],
                            lhsT=kt[hp:hp + 64, gg, 128 * j:128 * (j + 1)],
                            rhs=qt[hp:hp + 64, gg,
                                   512 * qc + vs:512 * (qc + 1)],
                            start=True, stop=True, skip_group_check=True)
                    etp = sb.tile([128, 2, 512], BF16, tag=f"et{sweep}",
                                  bufs=3)
                    nc.scalar.activation(etp[:, :, vs:512], scp[:, :, vs:512],
                                         EXP, scale=SCALE)
                    if j >= 4 * qc:
                        for hh in range(2):
                            nc.vector.tensor_mul(etp[:, hh, vs:vs + 128],
                                                 etp[:, hh, vs:vs + 128],
                                                 tri_t)
                    pend.append((j, etp))
                    if len(pend) > 1:   # lag-1 AV pipeline
                        do_av(*pend.pop(0))
                    slot = sweep * njt + j
                    k0 = nfl * slot // (2 * njt)
                    k1 = nfl * (slot + 1) // (2 * njt)
                    for k in range(k0, k1):
                        fillers[k]()
                for args in pend:
                    do_av(*args)
                # drain AV psum at the sweep boundary (frees the av banks)
                for hh in range(2):
                    raw = sb.tile([65, 512], F32R, tag="raw", bufs=8,
                                  name=f"raw{qc}_{h0 + hh}")
                    if qc == 3 and sweep == 1:
                        nc.scalar.activation(raw, avs[hh][0:65, :], COPY)
                    else:
                        nc.vector.tensor_copy(raw, avs[hh][0:65, :])
                    raw_of.setdefault(qc, []).append(raw)

        # ---- schedule ----
        # pre-phase: just enough projections for attention(0)
        qk_chain(wq_t, qt, 0, 0)
        qk_chain(wq_t, qt, 1, 0)
        qk_chain(wk_t, kt, 0, 0)
        qk_chain(wk_t, kt, 1, 0)
        for j in range(4):
            v_chain(j)

        attention(0, [
            lambda: qk_chain(wk_t, kt, 0, 1),
            lambda: qk_chain(wk_t, kt, 1, 1),
            lambda: qk_chain(wq_t, qt, 0, 1),
            lambda: qk_chain(wq_t, qt, 1, 1),
            lambda: v_chain(4),
            lambda: v_chain(5),
        ])
        attention(1, [
            lambda: v_chain(6),
            lambda: v_chain(7),
            lambda: qk_chain(wk_t, kt, 0, 2),
            lambda: qk_chain(wk_t, kt, 1, 2),
            lambda: part1_norm(0),
            lambda: qk_chain(wq_t, qt, 0, 2),
            lambda: qk_chain(wq_t, qt, 1, 2),
            lambda: v_chain(8),
            lambda: part2_unit(0, 0),
            lambda: part2_unit(0, 1),
            lambda: part2_unit(0, 2),
        ])
        attention(2, [
            lambda: v_chain(9),
            lambda: v_chain(10),
            lambda: v_chain(11),
            lambda: qk_chain(wk_t, kt, 0, 3),
            lambda: qk_chain(wk_t, kt, 1, 3),
            lambda: part2_unit(0, 3),
            lambda: part1_norm(1),
            lambda: qk_chain(wq_t, qt, 0, 3),
            lambda: qk_chain(wq_t, qt, 1, 3),
            lambda: part2_unit(1, 0),
            lambda: part2_unit(1, 1),
            lambda: v_chain(12),
        ])
        attention(3, [
            lambda: v_chain(13),
            lambda: v_chain(14),
            lambda: v_chain(15),
            lambda: part2_unit(1, 2),
            lambda: part2_unit(1, 3),
            lambda: part1_norm(2),
            lambda: part2_unit(2, 0),
            lambda: part2_unit(2, 1),
            lambda: part2_unit(2, 2),
            lambda: part2_unit(2, 3),
        ])
        # tail: normalize qc=3 in 128-column chunks so each O-projection
        # token tile starts as soon as its slice of the pairs is ready
        raws = raw_of[3]
        pairs = [sb.tile([128, 512], BF16, tag="pair", bufs=4,
                         name=f"pair3_{p}") for p in range(2)]
        rcps = []
        for h in range(NH):
            bc = ps.tile([64, 512], F32, tag="mm", bufs=2, name=f"bc3_{h}")
            nc.tensor.matmul(bc, lhsT=onesr_t[64:65, 0:64],
                             rhs=raws[h][64:65, :],
                             start=True, stop=True, skip_group_check=True)
            rcp = sb.tile([64, 512], F32, tag="rcp", bufs=4,
                          name=f"rcp3_{h}")
            nc.vector.reciprocal_approx_fast(rcp, bc)
            rcps.append(rcp)
        pair_of[3] = pairs
        for tt in range(4):
            cs = slice(128 * tt, 128 * (tt + 1))
            for h in range(NH):
                if h % 2 == 0:
                    nc.vector.tensor_mul(pairs[h // 2][0:64, cs],
                                         raws[h][0:64, cs], rcps[h][:, cs])
                else:
                    tmp = sb.tile([64, 128], BF16, tag="tmp3", bufs=4,
                                  name=f"tmp3_{h}_{tt}")
                    nc.vector.tensor_mul(tmp, raws[h][0:64, cs],
                                         rcps[h][:, cs])
                    nc.scalar.dma_start(pairs[h // 2][64:128, cs], tmp)
            part2_unit(3, tt)
    nc.compile()
    return nc


def _get_built():
    global _BUILT
    if _BUILT is None:
        _BUILT = _build()
    return _BUILT


def _host_inputs(x, q_proj, k_proj, v_proj, o_proj):
    bf = ml_dtypes.bfloat16
    # [p, tc, k, c] = x[b][512tc+c, 128k+p]
    xt = [np.ascontiguousarray(
        x[b].T.reshape(8, 128, 4, 512).transpose(1, 2, 0, 3).astype(bf))
        for b in range(B)]
    tri = np.triu(np.ones((128, 128), dtype=np.float32)).astype(bf)
    onesb = np.ones((128, 64), dtype=np.float32).astype(bf)
    onesr = np.ones((1, 64), dtype=np.float32)

    def wqk(w, g):
        # [fp, 1024*gg + 128*k + m] = w[256g+128gg+m, 128k+fp]
        a = w[256 * g:256 * (g + 1)].reshape(2, 128, 8, 128)
        return np.ascontiguousarray(
            a.transpose(3, 0, 2, 1).reshape(128, 2048).astype(bf))

    def wvt(w, g):
        # [fp, 256*k + vd] = w[256g+vd, 128k+fp]
        a = w[256 * g:256 * (g + 1)].reshape(256, 8, 128)
        return np.ascontiguousarray(
            a.transpose(2, 1, 0).reshape(128, 2048).astype(bf))

    def wo_s(w, g):
        # [dp, 1024*dd + o] = o_proj[o, 256g+128dd+dp]
        a = w[:, 256 * g:256 * (g + 1)].reshape(1024, 2, 128)
        return np.ascontiguousarray(
            a.transpose(2, 1, 0).reshape(128, 2048).astype(bf))

    in_maps = []
    for c in range(NCORES):
        b, g = c // 4, c % 4
        in_maps.append(dict(
            xt=xt[b], wq=wqk(q_proj, g), wk=wqk(k_proj, g),
            wvt=wvt(v_proj, g), wo=wo_s(o_proj, g), tri=tri,
            onesb=onesb, onesr=onesr))
    return in_maps


def kernel(**inputs):
    x = np.asarray(inputs["x"], dtype=np.float32)
    q_proj = np.asarray(inputs["q_proj"], dtype=np.float32)
    k_proj = np.asarray(inputs["k_proj"], dtype=np.float32)
    v_proj = np.asarray(inputs["v_proj"], dtype=np.float32)
    o_proj = np.asarray(inputs["o_proj"], dtype=np.float32)

    in_maps = _host_inputs(x, q_proj, k_proj, v_proj, o_proj)
    nc = _get_built()
    global LAST_RESULTS
    LAST_RESULTS = run_bass_kernel_spmd(
        nc, in_maps, core_ids=list(range(NCORES)),
        trace=bool(os.environ.get("KERNEL_TRACE")))
    out = np.zeros((B, S, D), dtype=np.float32)
    for c in range(NCORES):
        out[c // 4] += np.asarray(
            LAST_RESULTS.results[c]["out"]).astype(np.float32)
    return out


# revision 26
# speedup vs baseline: 1.6806x; 1.0355x over previous
"""Multi-head self-attention (B=2, S=2048, D=1024, H=16, causal) on 8 trn2 cores.

Sharding: core c = (batch b=c//4, head-group g=c%4 of 4 heads = dims
256g:256g+256). Column-parallel QKV, row-parallel O; each core returns a
partial [2048, 1024] output for its batch; host sums 4 partials per batch.

Per-core kernel (all matmul inputs bf16):
  - projections from xt [1024, 2048] bf16: QT/KT [128, 2, 2048] bf16;
    V computed directly in token-major layout vg[128, 16, 4, 65] with a
    ones column per head so AV also produces the softmax denominator.
  - attention in transposed-score layout: scoresT[k, q] = K @ Q^T tiles,
    exp on ACT (scale 1/8 fused) to bf16, causal staircase skips invalid
    columns, triangular mask on diagonal blocks only. Lag-1 AV pipeline.
  - software pipeline: only the first token-chunk of Q/K and V tiles 0-3
    are projected up front; the remaining projection chains run as PE
    fillers inside the attention j-loops (which are paced by the exp
    stream on ACT), as do the deferred normalize + O-projection units.
  - attention runs two sweeps of 2 heads per q-chunk so the exp covers
    a [128, 2, 512] two-bank PSUM pair in one ACT instruction (halves
    exp instruction count and ACT semaphore waits).
  - normalize: raw AV copy to SBUF (frees PSUM), denominator broadcast
    via f32r ones outer-product matmul, reciprocal_approx_fast on DVE
    (no Ln -> no ACT table switches), multiply to bf16 pairs; odd heads
    DMA-shifted to partitions 64:128 so O contracts 128 dims per pair.
  - O projection: 2 chained matmuls per 512-col chunk, single output
    DMA per 128-token tile; the qc=3 normalize is chunked by token tile
    so the tail O units start as early as possible.
"""

import os
import numpy as np
from contextlib import ExitStack

import ml_dtypes

import concourse.bass as bass
import concourse.tile as tile
from concourse import bacc, mybir
from concourse.bass_utils import run_bass_kernel_spmd

F32R = mybir.dt.float32r
F32 = mybir.dt.float32
BF16 = mybir.dt.bfloat16
EXP = mybir.ActivationFunctionType.Exp
COPY = mybir.ActivationFunctionType.Copy

B, S, D = 2, 2048, 1024
NCORES = 8
SCALE = 0.125         # 1/sqrt(64)
NH = 4                # heads per core

_BUILT = None
LAST_RESULTS = None


def _build():
    nc = bacc.Bacc("TRN2", target_bir_lowering=False, debug=False,
                   num_devices=NCORES)
    # xt packed [p, tc, k, c]: xt[p, tc, k, c] = x[b][512tc+c, 128k+p]
    xt_d = nc.dram_tensor("xt", [128, 4, 8, 512], BF16,
                          kind="ExternalInput").ap()
    wq_d = nc.dram_tensor("wq", [128, 2048], BF16, kind="ExternalInput").ap()
    wk_d = nc.dram_tensor("wk", [128, 2048], BF16, kind="ExternalInput").ap()
    wvt_d = nc.dram_tensor("wvt", [128, 2048], BF16, kind="ExternalInput").ap()
    wo_d = nc.dram_tensor("wo", [128, 2048], BF16, kind="ExternalInput").ap()
    tri_d = nc.dram_tensor("tri", [128, 128], BF16, kind="ExternalInput").ap()
    onesb_d = nc.dram_tensor("onesb", [128, 64], BF16,
                             kind="ExternalInput").ap()
    onesr_d = nc.dram_tensor("onesr", [1, 64], F32R, kind="ExternalInput").ap()
    out_d = nc.dram_tensor("out", [S, D], BF16, kind="ExternalOutput").ap()

    with tile.TileContext(nc) as tc, ExitStack() as ctx:
        consts = ctx.enter_context(tc.tile_pool(name="consts", bufs=1))
        sb = ctx.enter_context(tc.tile_pool(name="sb", bufs=1))
        ps = ctx.enter_context(tc.tile_pool(name="ps", bufs=1, space="PSUM"))

        # ---- DMA program: weights on the ACT hwdge queue, x on sync ----
        # (halves so the first projection chains start sooner)
        wq_t = consts.tile([128, 2048], BF16, tag="wq")
        nc.scalar.dma_start(wq_t[:, 0:1024], wq_d[:, 0:1024])
        nc.scalar.dma_start(wq_t[:, 1024:2048], wq_d[:, 1024:2048])
        wk_t = consts.tile([128, 2048], BF16, tag="wk")
        nc.scalar.dma_start(wk_t[:, 0:1024], wk_d[:, 0:1024])
        nc.scalar.dma_start(wk_t[:, 1024:2048], wk_d[:, 1024:2048])
        wvt_t = consts.tile([128, 2048], BF16, tag="wvt")
        nc.scalar.dma_start(wvt_t, wvt_d)
        wo_t = consts.tile([128, 2048], BF16, tag="wo")
        nc.scalar.dma_start(wo_t, wo_d)
        # x: one big contiguous DMA per token-chunk (tc0 split for latency)
        xts = []
        for tc2 in range(4):
            xk = sb.tile([128, 8, 512], BF16, tag="xts", bufs=4,
                         name=f"xts{tc2}")
            if tc2 == 0:
                for kq in range(4):
                    nc.sync.dma_start(xk[:, 2 * kq:2 * kq + 2, :],
                                      xt_d[:, 0, 2 * kq:2 * kq + 2, :])
            else:
                nc.sync.dma_start(xk, xt_d[:, tc2, :, :])
            xts.append(xk)
        tri_t = consts.tile([128, 128], BF16, tag="tri")
        nc.gpsimd.dma_start(tri_t, tri_d)
        onesb_t = consts.tile([128, 64], BF16, tag="onesb")
        nc.gpsimd.dma_start(onesb_t, onesb_d)
        # ones row at partition 64 (same base partition as the den row)
        onesr_t = consts.tile([65, 64], F32R, tag="onesr")
        nc.gpsimd.dma_start(onesr_t[64:65, :], onesr_d)

        qt = sb.tile([128, 2, 2048], BF16, tag="qt")
        kt = sb.tile([128, 2, 2048], BF16, tag="kt")
        vg = sb.tile([128, 16, NH, 65], BF16, tag="vg")
        # strided ones fill on the Pool engine (a strided DMA here would
        # degenerate to 2-byte SWDGE packets)
        nc.gpsimd.tensor_copy(vg[:, :, :, 64:65], onesb_t[:, 0:64])

        def qk_chain(w_t, dst, gg, tc2):
            pp = ps.tile([128, 512], F32, tag="mm", bufs=2)
            for k in range(8):
                nc.tensor.matmul(
                    pp,
                    lhsT=w_t[:, 1024 * gg + 128 * k:1024 * gg + 128 * (k + 1)],
                    rhs=xts[tc2][:, k, :],
                    start=(k == 0), stop=(k == 7), skip_group_check=True)
            nc.vector.tensor_copy(dst[:, gg, 512 * tc2:512 * (tc2 + 1)], pp)

        def v_chain(j):
            vp = ps.tile([128, 256], F32, tag="mm", bufs=2)
            for k in range(8):
                nc.tensor.matmul(
                    vp,
                    lhsT=xts[j // 4][:, k, 128 * (j % 4):128 * (j % 4 + 1)],
                    rhs=wvt_t[:, 256 * k:256 * (k + 1)],
                    start=(k == 0), stop=(k == 7), skip_group_check=True)
            nc.vector.tensor_copy(vg[:, j, :, 0:64], vp)

        # ---- deferred normalize (part1) / O projection (part2) ----
        raw_of = {}
        pair_of = {}

        def part1_norm(qc):
            raws = raw_of[qc]
            pairs = [sb.tile([128, 512], BF16, tag="pair", bufs=4,
                             name=f"pair{qc}_{p}") for p in range(2)]
            for h in range(NH):
                bc = ps.tile([64, 512], F32, tag="mm", bufs=2,
                             name=f"bc{qc}_{h}")
                nc.tensor.matmul(bc, lhsT=onesr_t[64:65, 0:64],
                                 rhs=raws[h][64:65, :],
                                 start=True, stop=True,
                                 skip_group_check=True)
                rcp = sb.tile([64, 512], F32, tag="rcp", bufs=4,
                              name=f"rcp{qc}_{h}")
                nc.vector.reciprocal_approx_fast(rcp, bc)
                if h % 2 == 0:
                    nc.vector.tensor_mul(pairs[h // 2][0:64, :],
                                         raws[h][0:64, :], rcp)
                else:
                    tmp = sb.tile([64, 512], BF16, tag="tmp", bufs=2,
                                  name=f"tmp{qc}_{h}")
                    nc.vector.tensor_mul(tmp, raws[h][0:64, :], rcp)
                    nc.scalar.dma_start(pairs[h // 2][64:128, :], tmp)
            pair_of[qc] = pairs

        def part2_unit(qc, tt):
            pairs = pair_of[qc]
            row0 = 512 * qc + 128 * tt
            ost = sb.tile([128, 1024], BF16, tag="ost", bufs=3,
                          name=f"ost{qc}_{tt}")
            for chv in range(2):
                op = ps.tile([128, 512], F32, tag="mm", bufs=2,
                             name=f"op{qc}_{tt}_{chv}")
                nc.tensor.matmul(
                    op, lhsT=pairs[0][:, 128 * tt:128 * (tt + 1)],
                    rhs=wo_t[:, 512 * chv:512 * (chv + 1)],
                    start=True, stop=False, skip_group_check=True)
                nc.tensor.matmul(
                    op, lhsT=pairs[1][:, 128 * tt:128 * (tt + 1)],
                    rhs=wo_t[:, 1024 + 512 * chv:1024 + 512 * (chv + 1)],
                    start=False, stop=True, skip_group_check=True)
                if chv == 0:
                    nc.vector.tensor_copy(ost[:, 0:512], op)
                else:
                    nc.scalar.activation(ost[:, 512:1024], op, COPY)
            nc.sync.dma_start(out_d[row0:row0 + 128, :], ost)

        # ---- attention: two sweeps of 2 heads; 1024-wide exp per j ----
        def attention(qc, fillers):
            njt = 4 * qc + 4
            nfl = len(fillers)
            for sweep in range(2):
                h0 = 2 * sweep
                avs = [ps.tile([128, 512], F32, tag="av", bufs=2,
                               name=f"avps{qc}_{sweep}_{hh}")
                       for hh in range(2)]
                pend = []

                def do_av(j, etp, avs=avs, h0=h0):
                    vs = max(0, 128 * (j - 4 * qc))
                    for hh in range(2):
                        nc.tensor.matmul(
                            avs[hh][0:65, vs:512],
                            lhsT=vg[:, j, h0 + hh, 0:65],
                            rhs=etp[:, hh, vs:512],
                            start=(j == 0), stop=(j == njt - 1),
                            skip_group_check=True)

                for j in range(njt):
                    vs = max(0, 128 * (j - 4 * qc))
                    scp = ps.tile([128, 2, 512], F32, tag="mm2", bufs=2)
                    for hh in range(2):
                        h = h0 + hh
                        hp = 64 * (h % 2)
                        gg = h // 2
                        nc.tensor.matmul(
                            scp[:, hh, vs:512],
                            lhsT=kt[hp:hp + 64, gg, 128 * j:128 * (j + 1)],
                            rhs=qt[hp:hp + 64, gg,
                                   512 * qc + vs:512 * (qc + 1)],
                            start=True, stop=True, skip_group_check=True)
                    etp = sb.tile([128, 2, 512], BF16, tag=f"et{sweep}",
                                  bufs=3)
                    nc.scalar.activation(etp[:, :, vs:512], scp[:, :, vs:512],
                                         EXP, scale=SCALE)
                    if j >= 4 * qc:
                        for hh in range(2):
                            nc.vector.tensor_mul(etp[:, hh, vs:vs + 128],
                                                 etp[:, hh, vs:vs + 128],
                                                 tri_t)
                    pend.append((j, etp))
                    if len(pend) > 1:   # lag-1 AV pipeline
                        do_av(*pend.pop(0))
                    slot = sweep * njt + j
                    k0 = nfl * slot // (2 * njt)
                    k1 = nfl * (slot + 1) // (2 * njt)
                    for k in range(k0, k1):
                        fillers[k]()
                for args in pend:
                    do_av(*args)
                # drain AV psum at the sweep boundary (frees the av banks)
                for hh in range(2):
                    raw = sb.tile([65, 512], F32R, tag="raw", bufs=8,
                                  name=f"raw{qc}_{h0 + hh}")
                    if qc == 3 and sweep == 1:
                        nc.scalar.activation(raw, avs[hh][0:65, :], COPY)
                    else:
                        nc.vector.tensor_copy(raw, avs[hh][0:65, :])
                    raw_of.setdefault(qc, []).append(raw)

        # ---- schedule ----
        # pre-phase: just enough projections for attention(0)
        qk_chain(wq_t, qt, 0, 0)
        qk_chain(wq_t, qt, 1, 0)
        qk_chain(wk_t, kt, 0, 0)
        qk_chain(wk_t, kt, 1, 0)
        for j in range(4):
            v_chain(j)

        attention(0, [
            lambda: qk_chain(wk_t, kt, 0, 1),
            lambda: qk_chain(wk_t, kt, 1, 1),
            lambda: qk_chain(wq_t, qt, 0, 1),
            lambda: qk_chain(wq_t, qt, 1, 1),
            lambda: v_chain(4),
            lambda: v_chain(5),
        ])
        attention(1, [
            lambda: v_chain(6),
            lambda: v_chain(7),
            lambda: qk_chain(wk_t, kt, 0, 2),
            lambda: qk_chain(wk_t, kt, 1, 2),
            lambda: part1_norm(0),
            lambda: qk_chain(wq_t, qt, 0, 2),
            lambda: qk_chain(wq_t, qt, 1, 2),
            lambda: v_chain(8),
            lambda: part2_unit(0, 0),
            lambda: part2_unit(0, 1),
            lambda: part2_unit(0, 2),
        ])
        attention(2, [
            lambda: v_chain(9),
            lambda: v_chain(10),
            lambda: v_chain(11),
            lambda: qk_chain(wk_t, kt, 0, 3),
            lambda: qk_chain(wk_t, kt, 1, 3),
            lambda: part2_unit(0, 3),
            lambda: part1_norm(1),
            lambda: qk_chain(wq_t, qt, 0, 3),
            lambda: qk_chain(wq_t, qt, 1, 3),
            lambda: part2_unit(1, 0),
            lambda: part2_unit(1, 1),
            lambda: v_chain(12),
        ])
        attention(3, [
            lambda: v_chain(13),
            lambda: v_chain(14),
            lambda: v_chain(15),
            lambda: part2_unit(1, 2),
            lambda: part2_unit(1, 3),
            lambda: part1_norm(2),
            lambda: part2_unit(2, 0),
            lambda: part2_unit(2, 1),
            lambda: part2_unit(2, 2),
            lambda: part2_unit(2, 3),
        ])
        # tail: normalize qc=3 in 128-column chunks so each O-projection
        # token tile starts as soon as its slice of the pairs is ready
        raws = raw_of[3]
        pairs = [sb.tile([128, 512], BF16, tag="pair", bufs=4,
                         name=f"pair3_{p}") for p in range(2)]
        rcps = []
        for h in range(NH):
            bc = ps.tile([64, 512], F32, tag="mm", bufs=2, name=f"bc3_{h}")
            nc.tensor.matmul(bc, lhsT=onesr_t[64:65, 0:64],
                             rhs=raws[h][64:65, :],
                             start=True, stop=True, skip_group_check=True)
            rcp = sb.tile([64, 512], F32, tag="rcp", bufs=4,
                          name=f"rcp3_{h}")
            nc.vector.reciprocal_approx_fast(rcp, bc)
            rcps.append(rcp)
        pair_of[3] = pairs
        for tt in range(4):
            cs = slice(128 * tt, 128 * (tt + 1))
            for h in range(NH):
                if h % 2 == 0:
                    nc.vector.tensor_mul(pairs[h // 2][0:64, cs],
                                         raws[h][0:64, cs], rcps[h][:, cs])
                else:
                    tmp = sb.tile([64, 128], BF16, tag="tmp3", bufs=4,
                                  name=f"tmp3_{h}_{tt}")
                    nc.vector.tensor_mul(tmp, raws[h][0:64, cs],
                                         rcps[h][:, cs])
                    nc.scalar.dma_start(pairs[h // 2][64:128, cs], tmp)
            part2_unit(3, tt)
    nc.compile()
    return nc


def _get_built():
    global _BUILT
    if _BUILT is None:
        _BUILT = _build()
    return _BUILT


def _host_inputs(x, q_proj, k_proj, v_proj, o_proj):
    bf = ml_dtypes.bfloat16
    # [p, tc, k, c] = x[b][512tc+c, 128k+p]
    xt = [np.ascontiguousarray(
        x[b].T.reshape(8, 128, 4, 512).transpose(1, 2, 0, 3).astype(bf))
        for b in range(B)]
    tri = np.triu(np.ones((128, 128), dtype=np.float32)).astype(bf)
    onesb = np.ones((128, 64), dtype=np.float32).astype(bf)
    onesr = np.ones((1, 64), dtype=np.float32)

    def wqk(w, g):
        # [fp, 1024*gg + 128*k + m] = w[256g+128gg+m, 128k+fp]
        a = w[256 * g:256 * (g + 1)].reshape(2, 128, 8, 128)
        return np.ascontiguousarray(
            a.transpose(3, 0, 2, 1).reshape(128, 2048).astype(bf))

    def wvt(w, g):
        # [fp, 256*k + vd] = w[256g+vd, 128k+fp]
        a = w[256 * g:256 * (g + 1)].reshape(256, 8, 128)
        return np.ascontiguousarray(
            a.transpose(2, 1, 0).reshape(128, 2048).astype(bf))

    def wo_s(w, g):
        # [dp, 1024*dd + o] = o_proj[o, 256g+128dd+dp]
        a = w[:, 256 * g:256 * (g + 1)].reshape(1024, 2, 128)
        return np.ascontiguousarray(
            a.transpose(2, 1, 0).reshape(128, 2048).astype(bf))

    in_maps = []
    for c in range(NCORES):
        b, g = c // 4, c % 4
        in_maps.append(dict(
            xt=xt[b], wq=wqk(q_proj, g), wk=wqk(k_proj, g),
            wvt=wvt(v_proj, g), wo=wo_s(o_proj, g), tri=tri,
            onesb=onesb, onesr=onesr))
    return in_maps


def kernel(**inputs):
    x = np.asarray(inputs["x"], dtype=np.float32)
    q_proj = np.asarray(inputs["q_proj"], dtype=np.float32)
    k_proj = np.asarray(inputs["k_proj"], dtype=np.float32)
    v_proj = np.asarray(inputs["v_proj"], dtype=np.float32)
    o_proj = np.asarray(inputs["o_proj"], dtype=np.float32)

    in_maps = _host_inputs(x, q_proj, k_proj, v_proj, o_proj)
    nc = _get_built()
    global LAST_RESULTS
    LAST_RESULTS = run_bass_kernel_spmd(
        nc, in_maps, core_ids=list(range(NCORES)),
        trace=bool(os.environ.get("KERNEL_TRACE")))
    out = np.zeros((B, S, D), dtype=np.float32)
    for c in range(NCORES):
        out[c // 4] += np.asarray(
            LAST_RESULTS.results[c]["out"]).astype(np.float32)
    return out
